# revision 1
# baseline (speedup 1.0000x reference)
"""ChebConv (K=3) GNN message passing on 8 Trainium2 NeuronCores.

Strategy (1D node partition, per sharding hint):
  - Nodes padded to NPAD rows and split into 8 contiguous blocks; core c owns
    dst rows [c*ROWS, (c+1)*ROWS) = WPC windows of 128 dst nodes each.
  - Both D^-1/2 scalings are folded into per-edge weights
    w_e = dinv[src] * dinv[dst], so each unnormalized-Laplacian application is
    a pure weighted segment-sum: h[dst] = sum_e w_e * x[src_e].
  - On device, each window's edges are processed as tiles of 128 edge slots:
    dma_gather fetches x[src] rows ([128 slots, 64] per tile), one fused DVE
    tensor_scalar builds the weighted one-hot lhsT ([slot, dst_local] =
    w * (iota == dstl)), and the tensor engine accumulates the window's
    segment-sum in PSUM across the window's tiles.
  - Chebyshev recurrence combines are node-local (ACT/DVE); X1 shards are
    exchanged between rounds via an on-device AllGather.
  - Edge slots are split into low/high src halves (gather indices are int16)
    and padded to uniform per-window tile counts (GL low + GH high) so the
    same NEFF runs on all 8 cores; per-core variation lives in input tables.
"""

import sys

for _p in ("/opt/trn_rl_repo",):
    if _p not in sys.path:
        sys.path.insert(0, _p)

import numpy as np

# Problem shape (hardcoded per contract).
N, E, D = 60000, 1200000, 64
NCORES = 8
WIN = 128           # dst nodes per window (PSUM partition dim)
WPC = 60            # windows per core
ROWS = WPC * WIN    # 7680 dst rows per core
NPAD = NCORES * ROWS  # 61440 padded node rows
SPLIT = 32768       # int16 gather index range per source half
CW = 6              # windows per gather chunk (WPC % CW == 0)
BF16 = True         # bf16 gather rows + one-hot (4x DVE mode, PE FWL)


def _preprocess(src, dst, w_e):
    """Build per-core gather/one-hot tables.

    Returns (GL, GH, tables) where tables[c] = dict(idx16, dstl, wts).
    """
    src = np.asarray(src, dtype=np.int64)
    dst = np.asarray(dst, dtype=np.int64)
    w_e = np.asarray(w_e, dtype=np.float32)

    gwin = dst // WIN          # global window id
    core = gwin // WPC
    wloc = gwin % WPC
    low = src < SPLIT

    # Per (core, window, half) tile counts -> uniform GL / GH.
    nlo = np.zeros((NCORES, WPC), np.int64)
    nhi = np.zeros((NCORES, WPC), np.int64)
    np.add.at(nlo, (core[low], wloc[low]), 1)
    np.add.at(nhi, (core[~low], wloc[~low]), 1)
    GL = int(np.max(np.ceil(nlo / 128)))
    GH = int(np.max(np.ceil(nhi / 128)))
    G = GL + GH
    TILES = WPC * G
    SLOTS = TILES * 128
    HI0 = WPC * GL * 128  # high-stream slot offset

    # Sort edges by (core, window, high-flag), then by src within each run
    # (src-sorted gather streams improve HBM row locality).
    key = (core * WPC + wloc) * 2 + (~low)
    order = np.argsort(key * (1 << 17) + src, kind="stable")
    s_src = src[order]
    s_w = w_e[order]
    s_dstl = (dst[order] % WIN).astype(np.float32)
    s_low = low[order]

    # Start offset of each (core, window, half) run in the sorted arrays.
    counts = np.zeros(NCORES * WPC * 2, np.int64)
    np.add.at(counts, key, 1)
    starts = np.concatenate([[0], np.cumsum(counts)])

    tables = []
    for c in range(NCORES):
        idx_slot = np.zeros(SLOTS, np.int16)
        w_slot = np.zeros(SLOTS, np.float32)
        dstl_slot = np.zeros(SLOTS, np.float32)
        for w in range(WPC):
            kbase = (c * WPC + w) * 2
            # low half
            a, b = starts[kbase], starts[kbase + 1]
            n = b - a
            o = w * GL * 128
            if n:
                assert s_low[a:b].all()
                idx_slot[o:o + n] = s_src[a:b].astype(np.int16)
                w_slot[o:o + n] = s_w[a:b]
                dstl_slot[o:o + n] = s_dstl[a:b]
            # high half
            a, b = starts[kbase + 1], starts[kbase + 2]
            n = b - a
            o = HI0 + w * GH * 128
            if n:
                assert not s_low[a:b].any()
                idx_slot[o:o + n] = (s_src[a:b] - SPLIT).astype(np.int16)
                w_slot[o:o + n] = s_w[a:b]
                dstl_slot[o:o + n] = s_dstl[a:b]

        # idx16 layout: stream pos i -> [i % 16, i // 16], replicated to
        # all 8 groups of 16 partitions (one per GPSIMD Q7 core).
        idx16 = np.tile(idx_slot.reshape(SLOTS // 16, 16).T, (8, 1))
        # Per-tile tables in STREAM tile order: [partition(slot%128), tile].
        dstl = dstl_slot.reshape(TILES, 128).T.copy()
        wts = w_slot.reshape(TILES, 128).T.copy()
        tables.append({"idx16": idx16, "dstl": dstl, "wts": wts})

    return GL, GH, tables


def _build_bass(GL, GH, re_norm):
    import concourse.bass as bass
    import concourse.bacc as bacc
    import concourse.mybir as mybir
    import concourse.tile as tile
    from contextlib import ExitStack

    f32 = mybir.dt.float32
    i16 = mybir.dt.int16
    bf16 = mybir.dt.bfloat16
    gdt = bf16 if BF16 else f32      # gather-row / one-hot dtype
    GC = 2 * D if BF16 else D        # gather row width (256B either way)
    AF = mybir.ActivationFunctionType
    OP = mybir.AluOpType

    G = GL + GH
    TILES = WPC * G
    SLOTS = TILES * 128
    HI0 = WPC * GL * 128

    a1 = float(-re_norm)            # X1 = a1*h1 + b1*X0
    b1 = float(re_norm - 1.0)
    a2 = float(-2.0 * re_norm)      # X2 = a2*h2 + b2*X1 - X0
    b2 = float(2.0 * (re_norm - 1.0))

    nc = bacc.Bacc(
        "TRN2",
        target_bir_lowering=False,
        debug=False,
        enable_asserts=False,
        num_devices=NCORES,
        num_swdge_queues=2,
    )
    xg1 = nc.dram_tensor("xg1", [NPAD, GC], gdt, kind="ExternalInput")
    x0own = nc.dram_tensor("x0own", [ROWS, D], f32, kind="ExternalInput")
    idx16_d = nc.dram_tensor("idx16", [128, SLOTS // 16], i16, kind="ExternalInput")
    dstl_d = nc.dram_tensor("dstl", [128, TILES], f32, kind="ExternalInput")
    wts_d = nc.dram_tensor("wts", [128, TILES], f32, kind="ExternalInput")
    out_d = nc.dram_tensor("out", [ROWS, 3 * D], f32, kind="ExternalOutput")
    import ml_dtypes
    _iota_np = np.broadcast_to(np.arange(128), (128, 128))
    iota_d = nc.inline_tensor(
        _iota_np.astype(ml_dtypes.bfloat16 if BF16 else np.float32),
        name="iota",
    )

    with ExitStack() as ctx:
        tc = ctx.enter_context(tile.TileContext(nc))
        dram = ctx.enter_context(tc.tile_pool(name="dram", bufs=1, space="DRAM"))
        x1shard = dram.tile([ROWS, GC], gdt)
        x1full = dram.tile([NPAD, GC], gdt, addr_space="Shared")

        cpool = ctx.enter_context(tc.tile_pool(name="const", bufs=1))
        idx_sb = cpool.tile([128, SLOTS // 16], i16)
        nc.sync.dma_start(out=idx_sb[:], in_=idx16_d[:])
        dstl_sb = cpool.tile([128, TILES], f32)
        nc.sync.dma_start(out=dstl_sb[:], in_=dstl_d[:])
        wts_sb = cpool.tile([128, TILES], f32)
        nc.sync.dma_start(out=wts_sb[:], in_=wts_d[:])
        iota_sb = cpool.tile([128, 128], gdt)
        nc.sync.dma_start(out=iota_sb[:], in_=iota_d[:])
        x0_sb = cpool.tile([128, WPC * D], f32)
        nc.sync.dma_start(
            out=x0_sb[:].rearrange("p (w d) -> p w d", d=D),
            in_=x0own[:].rearrange("(w p) d -> p w d", p=128),
        )
        x1_sb = cpool.tile([128, WPC * D], f32)

        gpool = ctx.enter_context(tc.tile_pool(name="gath", bufs=2))
        ohpool = ctx.enter_context(tc.tile_pool(name="oh", bufs=64))
        pspool = ctx.enter_context(tc.tile_pool(name="ps", bufs=8, space="PSUM"))
        mpool = ctx.enter_context(tc.tile_pool(name="misc", bufs=8))
        opool = ctx.enter_context(tc.tile_pool(name="outs", bufs=4))

        L_low = CW * GL * 128
        L_high = CW * GH * 128
        reg_low = nc.gpsimd.alloc_register("n_idx_low")
        nc.gpsimd.reg_mov(reg_low, L_low)
        if L_high != L_low:
            reg_high = nc.gpsimd.alloc_register("n_idx_high")
            nc.gpsimd.reg_mov(reg_high, L_high)
        else:
            reg_high = reg_low

        def do_round(xsrc, second):
            for wlo in range(0, WPC, CW):
                glow = gpool.tile([128, CW * GL, GC], gdt, tag="glow")
                ghigh = gpool.tile([128, CW * GH, GC], gdt, tag="ghigh")
                s0 = wlo * GL * 128
                L = CW * GL * 128
                nc.gpsimd.dma_gather(
                    out_ap=glow[:, :, :],
                    in_ap=xsrc[0:SPLIT, :],
                    idxs_ap=idx_sb[:, s0 // 16:(s0 + L) // 16],
                    num_idxs=L,
                    num_idxs_reg=reg_low,
                    elem_size=GC,
                    single_packet=False,
                )
                s0h = HI0 + wlo * GH * 128
                Lh = CW * GH * 128
                nc.gpsimd.dma_gather(
                    out_ap=ghigh[:, :, :],
                    in_ap=xsrc[SPLIT:NPAD, :],
                    idxs_ap=idx_sb[:, s0h // 16:(s0h + Lh) // 16],
                    num_idxs=Lh,
                    num_idxs_reg=reg_high,
                    elem_size=GC,
                    single_packet=False,
                    queue_num=1,
                )
                if second:
                    outc = opool.tile([128, CW * 3 * D], f32, tag="outc")
                else:
                    x1bf = None
                    if BF16:
                        x1bf = opool.tile([128, CW * GC], gdt, tag="x1bf",
                                          name="x1bf")
                        nc.gpsimd.memset(x1bf[:], 0)
                for wi in range(CW):
                    w = wlo + wi
                    ps = pspool.tile([128, D], f32)
                    for t in range(G):
                        if t < GL:
                            g_ap = glow[:, wi * GL + t, 0:D]
                            sti = w * GL + t
                        else:
                            g_ap = ghigh[:, wi * GH + (t - GL), 0:D]
                            sti = WPC * GL + w * GH + (t - GL)
                        oh = ohpool.tile([128, 128], gdt, tag="oh")
                        nc.vector.tensor_scalar(
                            out=oh[:],
                            in0=iota_sb[:],
                            scalar1=dstl_sb[:, sti:sti + 1],
                            scalar2=wts_sb[:, sti:sti + 1],
                            op0=OP.is_equal,
                            op1=OP.mult,
                        )
                        nc.tensor.matmul(
                            ps[:],
                            lhsT=oh[:],
                            rhs=g_ap,
                            start=(t == 0),
                            stop=(t == G - 1),
                        )
                    x0w = x0_sb[:, w * D:(w + 1) * D]
                    x1w = x1_sb[:, w * D:(w + 1) * D]
                    if not second:
                        # X1 = a1*h + b1*X0
                        tmp = mpool.tile([128, D], f32, tag="t1")
                        nc.scalar.activation(tmp[:], ps[:], AF.Copy, scale=a1)
                        if b1 == 1.0:
                            nc.vector.tensor_tensor(
                                out=x1w, in0=tmp[:], in1=x0w, op=OP.add
                            )
                        else:
                            xb = mpool.tile([128, D], f32, tag="t2")
                            nc.scalar.activation(xb[:], x0w, AF.Copy, scale=b1)
                            nc.vector.tensor_tensor(
                                out=x1w, in0=tmp[:], in1=xb[:], op=OP.add
                            )
                        if BF16:
                            nc.vector.tensor_copy(
                                out=x1bf[:, wi * GC:wi * GC + D], in_=x1w
                            )
                    else:
                        # X2 = a2*h + b2*X1 - X0
                        tmp = mpool.tile([128, D], f32, tag="t1")
                        nc.scalar.activation(tmp[:], ps[:], AF.Copy, scale=a2)
                        xb = mpool.tile([128, D], f32, tag="t2")
                        nc.scalar.activation(xb[:], x1w, AF.Copy, scale=b2)
                        t3 = mpool.tile([128, D], f32, tag="t3")
                        nc.vector.tensor_tensor(
                            out=t3[:], in0=tmp[:], in1=xb[:], op=OP.add
                        )
                        x2 = mpool.tile([128, D], f32, tag="t4")
                        nc.vector.tensor_tensor(
                            out=x2[:], in0=t3[:], in1=x0w, op=OP.subtract
                        )
                        # relu into the chunk output staging tile
                        ob = wi * 3 * D
                        nc.scalar.activation(outc[:, ob:ob + D], x0w, AF.Relu)
                        nc.scalar.activation(outc[:, ob + D:ob + 2 * D], x1w, AF.Relu)
                        nc.scalar.activation(outc[:, ob + 2 * D:ob + 3 * D], x2[:], AF.Relu)
                if not second:
                    # ship this chunk's X1 windows to the DRAM shard
                    if BF16:
                        nc.sync.dma_start(
                            out=x1shard[wlo * 128:(wlo + CW) * 128, :].rearrange(
                                "(w p) d -> p w d", p=128
                            ),
                            in_=x1bf[:].rearrange("p (w d) -> p w d", d=GC),
                        )
                    else:
                        nc.sync.dma_start(
                            out=x1shard[wlo * 128:(wlo + CW) * 128, :].rearrange(
                                "(w p) d -> p w d", p=128
                            ),
                            in_=x1_sb[:, wlo * D:(wlo + CW) * D].rearrange(
                                "p (w d) -> p w d", d=D
                            ),
                        )
                else:
                    nc.sync.dma_start(
                        out=out_d[wlo * 128:(wlo + CW) * 128, :].rearrange(
                            "(w p) d -> p w d", p=128
                        ),
                        in_=outc[:].rearrange("p (w d) -> p w d", d=3 * D),
                    )

        do_round(xg1, False)
        nc.gpsimd.collective_compute(
            "AllGather",
            mybir.AluOpType.bypass,
            replica_groups=[list(range(NCORES))],
            ins=[x1shard[:]],
            outs=[x1full[:]],
        )
        do_round(x1full, True)

    nc.finalize()
    return nc


def _make_in_maps(feat, src, dst, w_e):
    GL, GH, tables = _preprocess(src, dst, w_e)
    xpad = np.zeros((NPAD, D), np.float32)
    xpad[:N] = np.asarray(feat, np.float32)
    if BF16:
        import ml_dtypes
        xg = np.zeros((NPAD, 2 * D), ml_dtypes.bfloat16)
        xg[:, :D] = xpad.astype(ml_dtypes.bfloat16)
    else:
        xg = xpad
    in_maps = []
    for c in range(NCORES):
        t = tables[c]
        in_maps.append(
            {
                "xg1": xg,
                "x0own": xpad[c * ROWS:(c + 1) * ROWS],
                "idx16": t["idx16"],
                "dstl": t["dstl"],
                "wts": t["wts"],
            }
        )
    return GL, GH, in_maps


_CACHE = {}


def _get_program(feat, src, dst, lambda_max):
    re_norm = float(np.float32(2.0) / np.asarray(lambda_max, np.float32).reshape(-1)[0])
    deg = np.bincount(np.asarray(dst, np.int64), minlength=N).astype(np.float32)
    d_inv = np.maximum(deg, 1.0) ** -0.5
    src64 = np.asarray(src, np.int64)
    dst64 = np.asarray(dst, np.int64)
    w_e = (d_inv[src64] * d_inv[dst64]).astype(np.float32)
    GL, GH, in_maps = _make_in_maps(feat, src, dst, w_e)
    key = (GL, GH, re_norm)
    if key not in _CACHE:
        _CACHE[key] = _build_bass(GL, GH, re_norm)
    return _CACHE[key], in_maps


def kernel(feat, src, dst, lambda_max):
    from concourse.bass_utils import run_bass_kernel_spmd

    nc, in_maps = _get_program(feat, src, dst, lambda_max)
    res = run_bass_kernel_spmd(nc, in_maps, core_ids=list(range(NCORES)))
    kernel.last_exec_time_ns = res.exec_time_ns
    out = np.concatenate([res.results[c]["out"] for c in range(NCORES)], axis=0)
    return np.ascontiguousarray(out[:N])



# revision 3
# speedup vs baseline: 1.3152x; 1.3152x over previous
"""ChebConv (K=3) GNN message passing on 8 Trainium2 NeuronCores.

Strategy (1D node partition, per sharding hint):
  - Nodes padded to NPAD rows and split into 8 contiguous blocks; core c owns
    dst rows [c*ROWS, (c+1)*ROWS) = WPC windows of 128 dst nodes each.
  - Both D^-1/2 scalings are folded into per-edge weights
    w_e = dinv[src] * dinv[dst], so each unnormalized-Laplacian application is
    a pure weighted segment-sum: h[dst] = sum_e w_e * x[src_e].
  - On device, each window's edges are processed as tiles of 128 edge slots:
    dma_gather fetches x[src] rows ([128 slots, 64] per tile), one fused DVE
    tensor_scalar builds the weighted one-hot lhsT ([slot, dst_local] =
    w * (iota == dstl)), and the tensor engine accumulates the window's
    segment-sum in PSUM across the window's tiles.
  - Chebyshev recurrence combines are node-local (ACT/DVE); X1 shards are
    exchanged between rounds via an on-device AllGather.
  - Edge slots are split into low/high src halves (gather indices are int16)
    and padded to uniform per-window tile counts (GL low + GH high) so the
    same NEFF runs on all 8 cores; per-core variation lives in input tables.
"""

import sys

for _p in ("/opt/trn_rl_repo",):
    if _p not in sys.path:
        sys.path.insert(0, _p)

import numpy as np

# Problem shape (hardcoded per contract).
N, E, D = 60000, 1200000, 64
NCORES = 8
WIN = 128           # dst nodes per window (PSUM partition dim)
WPC = 60            # windows per core
ROWS = WPC * WIN    # 7680 dst rows per core
NPAD = NCORES * ROWS  # 61440 padded node rows
SPLIT = 32768       # int16 gather index range per source half
CW = 6              # windows per gather chunk (WPC % CW == 0)
BF16 = True         # bf16 gather rows + one-hot (4x DVE mode, PE FWL)


def _preprocess(src, dst, w_e):
    """Build per-core gather/one-hot tables.

    Returns (GL, GH, tables) where tables[c] = dict(idx16, dstl, wts).
    """
    src = np.asarray(src, dtype=np.int64)
    dst = np.asarray(dst, dtype=np.int64)
    w_e = np.asarray(w_e, dtype=np.float32)

    gwin = dst // WIN          # global window id
    core = gwin // WPC
    wloc = gwin % WPC
    low = src < SPLIT

    # Per (core, window, half) tile counts -> uniform GL / GH.
    nlo = np.zeros((NCORES, WPC), np.int64)
    nhi = np.zeros((NCORES, WPC), np.int64)
    np.add.at(nlo, (core[low], wloc[low]), 1)
    np.add.at(nhi, (core[~low], wloc[~low]), 1)
    GL = int(np.max(np.ceil(nlo / 128)))
    GH = int(np.max(np.ceil(nhi / 128)))
    G = GL + GH
    TILES = WPC * G
    SLOTS = TILES * 128
    HI0 = WPC * GL * 128  # high-stream slot offset

    # Sort edges by (core, window, high-flag), then by src within each run
    # (src-sorted gather streams improve HBM row locality).
    key = (core * WPC + wloc) * 2 + (~low)
    order = np.argsort(key * (1 << 17) + src, kind="stable")
    s_src = src[order]
    s_w = w_e[order]
    s_dstl = (dst[order] % WIN).astype(np.float32)
    s_low = low[order]

    # Start offset of each (core, window, half) run in the sorted arrays.
    counts = np.zeros(NCORES * WPC * 2, np.int64)
    np.add.at(counts, key, 1)
    starts = np.concatenate([[0], np.cumsum(counts)])

    tables = []
    for c in range(NCORES):
        idx_slot = np.zeros(SLOTS, np.int16)
        w_slot = np.zeros(SLOTS, np.float32)
        dstl_slot = np.zeros(SLOTS, np.float32)
        for w in range(WPC):
            kbase = (c * WPC + w) * 2
            # low half
            a, b = starts[kbase], starts[kbase + 1]
            n = b - a
            o = w * GL * 128
            if n:
                assert s_low[a:b].all()
                idx_slot[o:o + n] = s_src[a:b].astype(np.int16)
                w_slot[o:o + n] = s_w[a:b]
                dstl_slot[o:o + n] = s_dstl[a:b]
            # high half
            a, b = starts[kbase + 1], starts[kbase + 2]
            n = b - a
            o = HI0 + w * GH * 128
            if n:
                assert not s_low[a:b].any()
                idx_slot[o:o + n] = (s_src[a:b] - SPLIT).astype(np.int16)
                w_slot[o:o + n] = s_w[a:b]
                dstl_slot[o:o + n] = s_dstl[a:b]

        # idx16 layout: stream pos i -> [i % 16, i // 16], replicated to
        # all 8 groups of 16 partitions (one per GPSIMD Q7 core).
        idx16 = np.tile(idx_slot.reshape(SLOTS // 16, 16).T, (8, 1))
        # Per-tile tables in STREAM tile order: [partition(slot%128), tile].
        dstl = dstl_slot.reshape(TILES, 128).T.copy()
        wts = w_slot.reshape(TILES, 128).T.copy()
        tables.append({"idx16": idx16, "dstl": dstl, "wts": wts})

    return GL, GH, tables


def _dma_gather_thin(
    gp,
    out_ap,
    in_ap,
    idxs_ap,
    num_idxs,
    num_idxs_reg,
    elem_size,
    elem_step,
    queue_num=0,
    single_packet=False,
):
    """dma_gather with payload < 256B (elem_size*dtype need not be a 256B
    multiple); the source row stride (elem_step*dtype) still must be."""
    import concourse.mybir as mybir
    from concourse import ap_utils

    gp._assert_queue_num(queue_num)
    assert idxs_ap.dtype == mybir.dt.int16
    assert in_ap.dtype == out_ap.dtype
    assert in_ap.space.name == "DRAM"
    assert idxs_ap.space.name == "SBUF"
    assert out_ap.space.name == "SBUF"
    assert ap_utils.ap_is_contiguous(out_ap.ap[1:])
    assert ap_utils.ap_is_contiguous(idxs_ap.ap[1:])
    assert in_ap.ap[-1][1] == out_ap.ap[-1][1] == elem_size
    assert out_ap.ap[0][1] * out_ap.ap[1][1] == ((num_idxs + 127) // 128) * 128
    assert in_ap.ap[0][0] == elem_step
    stride_bytes = elem_step * mybir.dt.size(in_ap.dtype)
    assert stride_bytes % 256 == 0
    stride_bytes_256 = stride_bytes // 256
    assert stride_bytes_256 < 256

    _in_ap = gp.lower_ap_dma(in_ap, for_custom_bir_dma=True)
    _idxs_ap = gp.lower_ap(idxs_ap)
    _out_ap = gp.lower_ap(out_ap)
    return gp.add_instruction(
        mybir.InstDMAGatherAnt(
            name=gp.bass.get_next_instruction_name(),
            ins=[
                *_in_ap,
                _idxs_ap,
                gp.lower_val_access(gp.to_reg(num_idxs_reg)),
            ],
            outs=[_out_ap],
            transpose=False,
            num_idxs=num_idxs,
            elem_size=elem_size,
            stride_bytes_256=stride_bytes_256,
            gen_mode=0,
            single_packet=single_packet,
            queue_num=queue_num,
            sbuf_tokens_per_rank=0,
            sbuf_free_dim_per_rank=0,
            sbuf_free_dim_pad_per_rank=0,
            sbuf_byte_offset=0,
        )
    )


def _build_bass(GL, GH, re_norm):
    import concourse.bass as bass
    import concourse.bacc as bacc
    import concourse.mybir as mybir
    import concourse.tile as tile
    from contextlib import ExitStack

    f32 = mybir.dt.float32
    i16 = mybir.dt.int16
    bf16 = mybir.dt.bfloat16
    gdt = bf16 if BF16 else f32      # gather-row / one-hot dtype
    GC = 2 * D if BF16 else D        # gather row width (256B either way)
    AF = mybir.ActivationFunctionType
    OP = mybir.AluOpType

    G = GL + GH
    TILES = WPC * G
    SLOTS = TILES * 128
    HI0 = WPC * GL * 128

    a1 = float(-re_norm)            # X1 = a1*h1 + b1*X0
    b1 = float(re_norm - 1.0)
    a2 = float(-2.0 * re_norm)      # X2 = a2*h2 + b2*X1 - X0
    b2 = float(2.0 * (re_norm - 1.0))

    nc = bacc.Bacc(
        "TRN2",
        target_bir_lowering=False,
        debug=False,
        enable_asserts=False,
        num_devices=NCORES,
        num_swdge_queues=2,
    )
    xg1 = nc.dram_tensor("xg1", [NPAD, GC], gdt, kind="ExternalInput")
    x0own = nc.dram_tensor("x0own", [ROWS, D], f32, kind="ExternalInput")
    idx16_d = nc.dram_tensor("idx16", [128, SLOTS // 16], i16, kind="ExternalInput")
    dstl_d = nc.dram_tensor("dstl", [128, TILES], f32, kind="ExternalInput")
    wts_d = nc.dram_tensor("wts", [128, TILES], f32, kind="ExternalInput")
    out_d = nc.dram_tensor("out", [ROWS, 3 * D], f32, kind="ExternalOutput")
    import ml_dtypes
    _iota_np = np.broadcast_to(np.arange(128), (128, 128))
    iota_d = nc.inline_tensor(
        _iota_np.astype(ml_dtypes.bfloat16 if BF16 else np.float32),
        name="iota",
    )

    with ExitStack() as ctx:
        tc = ctx.enter_context(tile.TileContext(nc))
        dram = ctx.enter_context(tc.tile_pool(name="dram", bufs=1, space="DRAM"))
        x1shard = dram.tile([ROWS, GC], gdt)
        x1full = dram.tile([NPAD, GC], gdt, addr_space="Shared")

        cpool = ctx.enter_context(tc.tile_pool(name="const", bufs=1))
        idx_sb = cpool.tile([128, SLOTS // 16], i16)
        nc.sync.dma_start(out=idx_sb[:], in_=idx16_d[:])
        dstl_sb = cpool.tile([128, TILES], f32)
        nc.sync.dma_start(out=dstl_sb[:], in_=dstl_d[:])
        wts_sb = cpool.tile([128, TILES], f32)
        nc.sync.dma_start(out=wts_sb[:], in_=wts_d[:])
        iota_sb = cpool.tile([128, 128], gdt)
        nc.sync.dma_start(out=iota_sb[:], in_=iota_d[:])
        x0_sb = cpool.tile([128, WPC * D], f32)
        nc.sync.dma_start(
            out=x0_sb[:].rearrange("p (w d) -> p w d", d=D),
            in_=x0own[:].rearrange("(w p) d -> p w d", p=128),
        )
        x1_sb = cpool.tile([128, WPC * D], f32)

        gpool = ctx.enter_context(tc.tile_pool(name="gath", bufs=2))
        ohpool = ctx.enter_context(tc.tile_pool(name="oh", bufs=64))
        pspool = ctx.enter_context(tc.tile_pool(name="ps", bufs=8, space="PSUM"))
        mpool = ctx.enter_context(tc.tile_pool(name="misc", bufs=8))
        opool = ctx.enter_context(tc.tile_pool(name="outs", bufs=4))

        L_low = CW * GL * 128
        L_high = CW * GH * 128
        reg_low = nc.gpsimd.alloc_register("n_idx_low")
        nc.gpsimd.reg_mov(reg_low, L_low)
        if L_high != L_low:
            reg_high = nc.gpsimd.alloc_register("n_idx_high")
            nc.gpsimd.reg_mov(reg_high, L_high)
        else:
            reg_high = reg_low

        def do_round(xsrc, second):
            for wlo in range(0, WPC, CW):
                glow = gpool.tile([128, CW * GL, D], gdt, tag="glow")
                ghigh = gpool.tile([128, CW * GH, D], gdt, tag="ghigh")
                s0 = wlo * GL * 128
                L = CW * GL * 128
                _dma_gather_thin(
                    nc.gpsimd,
                    out_ap=glow[:, :, :],
                    in_ap=xsrc[0:SPLIT, 0:D],
                    idxs_ap=idx_sb[:, s0 // 16:(s0 + L) // 16],
                    num_idxs=L,
                    num_idxs_reg=reg_low,
                    elem_size=D,
                    elem_step=GC,
                )
                s0h = HI0 + wlo * GH * 128
                Lh = CW * GH * 128
                _dma_gather_thin(
                    nc.gpsimd,
                    out_ap=ghigh[:, :, :],
                    in_ap=xsrc[SPLIT:NPAD, 0:D],
                    idxs_ap=idx_sb[:, s0h // 16:(s0h + Lh) // 16],
                    num_idxs=Lh,
                    num_idxs_reg=reg_high,
                    elem_size=D,
                    elem_step=GC,
                    queue_num=1,
                )
                if second:
                    outc = opool.tile([128, CW * 3 * D], f32, tag="outc")
                else:
                    x1bf = None
                    if BF16:
                        x1bf = opool.tile([128, CW * GC], gdt, tag="x1bf",
                                          name="x1bf")
                        nc.gpsimd.memset(x1bf[:], 0)
                for wi in range(CW):
                    w = wlo + wi
                    ps = pspool.tile([128, D], f32)
                    for t in range(G):
                        if t < GL:
                            g_ap = glow[:, wi * GL + t, 0:D]
                            sti = w * GL + t
                        else:
                            g_ap = ghigh[:, wi * GH + (t - GL), 0:D]
                            sti = WPC * GL + w * GH + (t - GL)
                        oh = ohpool.tile([128, 128], gdt, tag="oh")
                        nc.vector.tensor_scalar(
                            out=oh[:],
                            in0=iota_sb[:],
                            scalar1=dstl_sb[:, sti:sti + 1],
                            scalar2=wts_sb[:, sti:sti + 1],
                            op0=OP.is_equal,
                            op1=OP.mult,
                        )
                        nc.tensor.matmul(
                            ps[:],
                            lhsT=oh[:],
                            rhs=g_ap,
                            start=(t == 0),
                            stop=(t == G - 1),
                        )
                    x0w = x0_sb[:, w * D:(w + 1) * D]
                    x1w = x1_sb[:, w * D:(w + 1) * D]
                    if not second:
                        # X1 = a1*h + b1*X0
                        tmp = mpool.tile([128, D], f32, tag="t1")
                        nc.scalar.activation(tmp[:], ps[:], AF.Copy, scale=a1)
                        if b1 == 1.0:
                            nc.vector.tensor_tensor(
                                out=x1w, in0=tmp[:], in1=x0w, op=OP.add
                            )
                        else:
                            xb = mpool.tile([128, D], f32, tag="t2")
                            nc.scalar.activation(xb[:], x0w, AF.Copy, scale=b1)
                            nc.vector.tensor_tensor(
                                out=x1w, in0=tmp[:], in1=xb[:], op=OP.add
                            )
                        if BF16:
                            nc.vector.tensor_copy(
                                out=x1bf[:, wi * GC:wi * GC + D], in_=x1w
                            )
                    else:
                        # X2 = a2*h + b2*X1 - X0
                        tmp = mpool.tile([128, D], f32, tag="t1")
                        nc.scalar.activation(tmp[:], ps[:], AF.Copy, scale=a2)
                        xb = mpool.tile([128, D], f32, tag="t2")
                        nc.scalar.activation(xb[:], x1w, AF.Copy, scale=b2)
                        t3 = mpool.tile([128, D], f32, tag="t3")
                        nc.vector.tensor_tensor(
                            out=t3[:], in0=tmp[:], in1=xb[:], op=OP.add
                        )
                        x2 = mpool.tile([128, D], f32, tag="t4")
                        nc.vector.tensor_tensor(
                            out=x2[:], in0=t3[:], in1=x0w, op=OP.subtract
                        )
                        # relu into the chunk output staging tile
                        ob = wi * 3 * D
                        nc.scalar.activation(outc[:, ob:ob + D], x0w, AF.Relu)
                        nc.scalar.activation(outc[:, ob + D:ob + 2 * D], x1w, AF.Relu)
                        nc.scalar.activation(outc[:, ob + 2 * D:ob + 3 * D], x2[:], AF.Relu)
                if not second:
                    # ship this chunk's X1 windows to the DRAM shard
                    if BF16:
                        nc.sync.dma_start(
                            out=x1shard[wlo * 128:(wlo + CW) * 128, :].rearrange(
                                "(w p) d -> p w d", p=128
                            ),
                            in_=x1bf[:].rearrange("p (w d) -> p w d", d=GC),
                        )
                    else:
                        nc.sync.dma_start(
                            out=x1shard[wlo * 128:(wlo + CW) * 128, :].rearrange(
                                "(w p) d -> p w d", p=128
                            ),
                            in_=x1_sb[:, wlo * D:(wlo + CW) * D].rearrange(
                                "p (w d) -> p w d", d=D
                            ),
                        )
                else:
                    nc.sync.dma_start(
                        out=out_d[wlo * 128:(wlo + CW) * 128, :].rearrange(
                            "(w p) d -> p w d", p=128
                        ),
                        in_=outc[:].rearrange("p (w d) -> p w d", d=3 * D),
                    )

        do_round(xg1, False)
        nc.gpsimd.collective_compute(
            "AllGather",
            mybir.AluOpType.bypass,
            replica_groups=[list(range(NCORES))],
            ins=[x1shard[:]],
            outs=[x1full[:]],
        )
        do_round(x1full, True)

    nc.finalize()
    return nc


def _make_in_maps(feat, src, dst, w_e):
    GL, GH, tables = _preprocess(src, dst, w_e)
    xpad = np.zeros((NPAD, D), np.float32)
    xpad[:N] = np.asarray(feat, np.float32)
    if BF16:
        import ml_dtypes
        xg = np.zeros((NPAD, 2 * D), ml_dtypes.bfloat16)
        xg[:, :D] = xpad.astype(ml_dtypes.bfloat16)
    else:
        xg = xpad
    in_maps = []
    for c in range(NCORES):
        t = tables[c]
        in_maps.append(
            {
                "xg1": xg,
                "x0own": xpad[c * ROWS:(c + 1) * ROWS],
                "idx16": t["idx16"],
                "dstl": t["dstl"],
                "wts": t["wts"],
            }
        )
    return GL, GH, in_maps


_CACHE = {}


def _get_program(feat, src, dst, lambda_max):
    re_norm = float(np.float32(2.0) / np.asarray(lambda_max, np.float32).reshape(-1)[0])
    deg = np.bincount(np.asarray(dst, np.int64), minlength=N).astype(np.float32)
    d_inv = np.maximum(deg, 1.0) ** -0.5
    src64 = np.asarray(src, np.int64)
    dst64 = np.asarray(dst, np.int64)
    w_e = (d_inv[src64] * d_inv[dst64]).astype(np.float32)
    GL, GH, in_maps = _make_in_maps(feat, src, dst, w_e)
    key = (GL, GH, re_norm)
    if key not in _CACHE:
        _CACHE[key] = _build_bass(GL, GH, re_norm)
    return _CACHE[key], in_maps


def kernel(feat, src, dst, lambda_max):
    from concourse.bass_utils import run_bass_kernel_spmd

    nc, in_maps = _get_program(feat, src, dst, lambda_max)
    res = run_bass_kernel_spmd(nc, in_maps, core_ids=list(range(NCORES)))
    kernel.last_exec_time_ns = res.exec_time_ns
    out = np.concatenate([res.results[c]["out"] for c in range(NCORES)], axis=0)
    return np.ascontiguousarray(out[:N])



# revision 11
# speedup vs baseline: 1.4961x; 1.1376x over previous
"""ChebConv (K=3) GNN message passing on 8 Trainium2 NeuronCores.

Strategy (v2):
  - Nodes are permuted into NPAD=61440 storage rows (8 blocks of ROWS=7680,
    60 windows of 128 per block) by a host-side greedy balancer that evens
    out per-window edge counts for both rounds (minimizes tile padding).
  - D^-1/2 normalization is folded into the gathered feature rows
    (x' = dinv*x, host-side for round 1, fused on-device for round 2) and a
    per-dst-node multiply after aggregation; one-hot lhsT tiles are pure 0/1
    indicators (padding slots get an out-of-range dst-local id -> zero col).
  - Round 1 groups edges by DST core (full x0 is resident on every core, so
    no communication): thin dma_gathers (128B payload / 256B stride) fetch
    x'[src] rows; per 6-window PSUM group, one-hot matmuls accumulate the
    segment-sum, then X1 = a1*dinv.h + b1*X0 node-locally.
  - Round 2 groups edges by SRC core (x1[src] is then core-local): each core
    computes partial h2 for ALL 480 global windows from its own X1 block,
    writes bf16 partials to DRAM, and a single ReduceScatter(add) both sums
    the partials and delivers each core its own dst block. The only
    collective is that RS (~8x smaller output than an X1 AllGather).
  - Post phase: X2 = a2*dinv.h2 + b2*X1 - X0, relu, concat, output.
"""

import sys

for _p in ("/opt/trn_rl_repo",):
    if _p not in sys.path:
        sys.path.insert(0, _p)

import numpy as np

# Problem shape (hardcoded per contract).
N, E, D = 60000, 1200000, 64
NCORES = 8
WIN = 128             # dst nodes per window (PSUM partition dim)
WPC = 60              # windows per core
ROWS = WPC * WIN      # 7680 rows per core block
NPAD = NCORES * ROWS  # 61440 padded node rows
NWIN = NCORES * WPC   # 480 global windows
SPLIT = 32768         # int16 gather index range per source half (round 1)
CW = 6                # round-1 windows per chunk (and PSUM group size)
CW2 = 24              # round-2 windows per chunk (4 PSUM groups of 6)
GRP = 6               # windows per PSUM group
PAD_DSTL = 300.0      # out-of-range dst-local id for padding slots


def _dma_gather_thin(
    gp,
    out_ap,
    in_ap,
    idxs_ap,
    num_idxs,
    num_idxs_reg,
    elem_size,
    elem_step,
    queue_num=0,
    single_packet=False,
):
    """dma_gather with payload < 256B (elem_size*dtype need not be a 256B
    multiple); the source row stride (elem_step*dtype) still must be."""
    import concourse.mybir as mybir
    from concourse import ap_utils

    gp._assert_queue_num(queue_num)
    assert idxs_ap.dtype == mybir.dt.int16
    assert in_ap.dtype == out_ap.dtype
    assert in_ap.space.name == "DRAM"
    assert idxs_ap.space.name == "SBUF"
    assert out_ap.space.name == "SBUF"
    assert ap_utils.ap_is_contiguous(out_ap.ap[1:])
    assert ap_utils.ap_is_contiguous(idxs_ap.ap[1:])
    assert in_ap.ap[-1][1] == out_ap.ap[-1][1] == elem_size
    assert out_ap.ap[0][1] * out_ap.ap[1][1] == ((num_idxs + 127) // 128) * 128
    assert in_ap.ap[0][0] == elem_step
    stride_bytes = elem_step * mybir.dt.size(in_ap.dtype)
    assert stride_bytes % 256 == 0
    stride_bytes_256 = stride_bytes // 256
    assert stride_bytes_256 < 256

    _in_ap = gp.lower_ap_dma(in_ap, for_custom_bir_dma=True)
    _idxs_ap = gp.lower_ap(idxs_ap)
    _out_ap = gp.lower_ap(out_ap)
    return gp.add_instruction(
        mybir.InstDMAGatherAnt(
            name=gp.bass.get_next_instruction_name(),
            ins=[
                *_in_ap,
                _idxs_ap,
                gp.lower_val_access(gp.to_reg(num_idxs_reg)),
            ],
            outs=[_out_ap],
            transpose=False,
            num_idxs=num_idxs,
            elem_size=elem_size,
            stride_bytes_256=stride_bytes_256,
            gen_mode=0,
            single_packet=single_packet,
            queue_num=queue_num,
            sbuf_tokens_per_rank=0,
            sbuf_free_dim_per_rank=0,
            sbuf_free_dim_pad_per_rank=0,
            sbuf_byte_offset=0,
        )
    )


def _balance_rows(src, dst):
    """Greedy node->storage-row permutation balancing per-window edge counts.

    Keeps each node in its natural core block; within a block distributes the
    7680 nodes over 60 windows to even out (a) in-edge counts split by src
    half (round-1 low/high gather streams) and (b) in-edge counts split by
    src core (round-2 partial aggregation cells).

    Returns row_of[NPAD]: natural node id -> storage row.
    """
    core_of_nat = np.minimum(np.arange(NPAD) // ROWS, NCORES - 1)
    src_core = np.minimum(src // ROWS, NCORES - 1)

    # provisional half classification by natural id (exact except for core-4
    # srcs whose final row may cross SPLIT; core 4 is balanced first so its
    # final rows are used for every other block)
    row_of = np.arange(NPAD, dtype=np.int64)

    def node_features(c, row_of):
        nodes = np.arange(c * ROWS, (c + 1) * ROWS, dtype=np.int64)
        # in-edges by src core: M[i, j] = #edges (src in core j) -> node i
        sel = np.minimum(dst // ROWS, NCORES - 1) == c
        d_loc = dst[sel] - c * ROWS
        F = np.zeros((ROWS, 10), np.int64)
        np.add.at(F, (d_loc, src_core[sel]), 1)
        low = row_of[src[sel]] < SPLIT
        np.add.at(F, (d_loc[low], 8), 1)
        np.add.at(F, (d_loc[~low], 9), 1)
        return nodes, F

    order_c = [4] + [c for c in range(NCORES) if c != 4]
    for c in order_c:
        nodes, F = node_features(c, row_of)
        tot = F.sum(axis=0).astype(np.float64)
        target = np.maximum(tot / WPC, 1.0)
        # process nodes by total degree, heaviest first
        deg_tot = F[:, :8].sum(axis=1)
        nd_order = np.argsort(-deg_tot, kind="stable")
        W = np.zeros((WPC, 10), np.float64)
        nfill = np.zeros(WPC, np.int64)
        win_of = np.zeros(ROWS, np.int64)
        Fn = F.astype(np.float64)
        for i in nd_order:
            score = ((W + Fn[i]) / target).max(axis=1)
            score[nfill >= WIN] = np.inf
            w = int(np.argmin(score))
            win_of[i] = w
            W[w] += Fn[i]
            nfill[w] += 1
        # rows: window-major, slot order by node id within window
        idx_sorted = np.lexsort((np.arange(ROWS), win_of))
        rows = c * ROWS + np.arange(ROWS)
        row_of[nodes[idx_sorted]] = rows
    return row_of


def _preprocess(src, dst):
    """Build balanced permutation + per-core gather/one-hot tables."""
    src = np.asarray(src, dtype=np.int64)
    dst = np.asarray(dst, dtype=np.int64)

    row_of = _balance_rows(src, dst)
    srcr = row_of[src]
    dstr = row_of[dst]

    gwin = dstr // WIN
    dcore = gwin // WPC
    wloc = gwin % WPC
    scoreid = srcr // ROWS
    low = srcr < SPLIT
    dstl = (dstr % WIN).astype(np.float32)

    # ---- Round 1 (dst-grouped) ----
    nlo = np.zeros((NCORES, WPC), np.int64)
    nhi = np.zeros((NCORES, WPC), np.int64)
    np.add.at(nlo, (dcore[low], wloc[low]), 1)
    np.add.at(nhi, (dcore[~low], wloc[~low]), 1)
    GL = int(np.max((nlo + 127) // 128))
    GH = int(np.max((nhi + 127) // 128))
    T1 = WPC * (GL + GH)
    S1 = T1 * 128
    HI0 = WPC * GL * 128

    key1 = (dcore * WPC + wloc) * 2 + (~low)
    order1 = np.argsort(key1 * (1 << 17) + srcr, kind="stable")
    s1_src = srcr[order1]
    s1_dstl = dstl[order1]
    counts1 = np.zeros(NCORES * WPC * 2, np.int64)
    np.add.at(counts1, key1, 1)
    starts1 = np.concatenate([[0], np.cumsum(counts1)])

    # ---- Round 2 (src-grouped over all 480 global windows) ----
    cnt2 = np.zeros((NCORES, NWIN), np.int64)
    np.add.at(cnt2, (scoreid, gwin), 1)
    G2 = int(np.max((cnt2 + 127) // 128))
    T2 = NWIN * G2
    S2 = T2 * 128

    key2 = scoreid * NWIN + gwin
    order2 = np.argsort(key2 * (1 << 13) + (srcr % ROWS), kind="stable")
    s2_src = (srcr % ROWS)[order2]
    s2_dstl = dstl[order2]
    counts2 = np.zeros(NCORES * NWIN, np.int64)
    np.add.at(counts2, key2, 1)
    starts2 = np.concatenate([[0], np.cumsum(counts2)])

    tables = []
    for c in range(NCORES):
        idx1 = np.zeros(S1, np.int16)
        dstl1 = np.full(S1, PAD_DSTL, np.float32)
        for w in range(WPC):
            kbase = (c * WPC + w) * 2
            a, b = starts1[kbase], starts1[kbase + 1]
            n = b - a
            o = w * GL * 128
            if n:
                idx1[o:o + n] = s1_src[a:b].astype(np.int16)
                dstl1[o:o + n] = s1_dstl[a:b]
            a, b = starts1[kbase + 1], starts1[kbase + 2]
            n = b - a
            o = HI0 + w * GH * 128
            if n:
                idx1[o:o + n] = (s1_src[a:b] - SPLIT).astype(np.int16)
                dstl1[o:o + n] = s1_dstl[a:b]

        idx2 = np.zeros(S2, np.int16)
        dstl2 = np.full(S2, PAD_DSTL, np.float32)
        for g in range(NWIN):
            k = c * NWIN + g
            a, b = starts2[k], starts2[k + 1]
            n = b - a
            o = g * G2 * 128
            if n:
                idx2[o:o + n] = s2_src[a:b].astype(np.int16)
                dstl2[o:o + n] = s2_dstl[a:b]

        tables.append(
            {
                "idx1": np.tile(idx1.reshape(S1 // 16, 16).T, (8, 1)),
                "dstl1": dstl1.reshape(T1, 128).T.copy(),
                "idx2": np.tile(idx2.reshape(S2 // 16, 16).T, (8, 1)),
                "dstl2": dstl2.reshape(T2, 128).T.copy(),
            }
        )
    return row_of, GL, GH, G2, tables


def _build_bass(GL, GH, G2, re_norm):
    import concourse.bass as bass
    import concourse.bacc as bacc
    import concourse.mybir as mybir
    import concourse.tile as tile
    import ml_dtypes
    from contextlib import ExitStack

    f32 = mybir.dt.float32
    i16 = mybir.dt.int16
    bf16 = mybir.dt.bfloat16
    AF = mybir.ActivationFunctionType
    OP = mybir.AluOpType

    G = GL + GH
    T1 = WPC * G
    S1 = T1 * 128
    HI0 = WPC * GL * 128
    T2 = NWIN * G2
    S2 = T2 * 128
    NGRP = WPC // GRP          # per-core window groups (10)

    a1 = float(-re_norm)       # X1 = a1*dinv.h1 + b1*X0
    b1 = float(re_norm - 1.0)
    a2 = float(-2.0 * re_norm)  # X2 = a2*dinv.h2 + b2*X1 - X0
    b2 = float(2.0 * (re_norm - 1.0))

    nc = bacc.Bacc(
        "TRN2",
        target_bir_lowering=False,
        debug=False,
        enable_asserts=False,
        num_devices=NCORES,
        num_swdge_queues=2,
    )
    xg = nc.dram_tensor("xg", [NPAD, 2 * D], bf16, kind="ExternalInput")
    x0own = nc.dram_tensor("x0own", [ROWS, D], f32, kind="ExternalInput")
    dinvb_d = nc.dram_tensor("dinvb", [128, WPC * D], f32, kind="ExternalInput")
    idx1_d = nc.dram_tensor("idx1", [128, S1 // 16], i16, kind="ExternalInput")
    dstl1_d = nc.dram_tensor("dstl1", [128, T1], f32, kind="ExternalInput")
    idx2_d = nc.dram_tensor("idx2", [128, S2 // 16], i16, kind="ExternalInput")
    dstl2_d = nc.dram_tensor("dstl2", [128, T2], f32, kind="ExternalInput")
    out_d = nc.dram_tensor("out", [ROWS, 3 * D], f32, kind="ExternalOutput")
    iota_d = nc.inline_tensor(
        np.broadcast_to(np.arange(128), (128, 128)).astype(ml_dtypes.bfloat16),
        name="iota",
    )

    with ExitStack() as ctx:
        tc = ctx.enter_context(tile.TileContext(nc))
        dram = ctx.enter_context(tc.tile_pool(name="dram", bufs=1, space="DRAM"))
        x1g = dram.tile([ROWS, 2 * D], bf16)
        hpart = dram.tile([NPAD, D], bf16)
        hrs = dram.tile([ROWS, D], bf16)

        cpool = ctx.enter_context(tc.tile_pool(name="const", bufs=1))
        idx1_sb = cpool.tile([128, S1 // 16], i16)
        nc.sync.dma_start(out=idx1_sb[:], in_=idx1_d[:])
        idx2_sb = cpool.tile([128, S2 // 16], i16)
        nc.sync.dma_start(out=idx2_sb[:], in_=idx2_d[:])
        dstl1_sb = cpool.tile([128, T1], f32)
        nc.sync.dma_start(out=dstl1_sb[:], in_=dstl1_d[:])
        dstl2_sb = cpool.tile([128, T2], f32)
        nc.sync.dma_start(out=dstl2_sb[:], in_=dstl2_d[:])
        iota_sb = cpool.tile([128, 128], bf16)
        nc.sync.dma_start(out=iota_sb[:], in_=iota_d[:])
        dinvb_sb = cpool.tile([128, WPC * D], f32)
        nc.sync.dma_start(out=dinvb_sb[:], in_=dinvb_d[:])
        x0_sb = cpool.tile([128, WPC * D], f32)
        nc.sync.dma_start(
            out=x0_sb[:].rearrange("p (w d) -> p w d", d=D),
            in_=x0own[:].rearrange("(w p) d -> p w d", p=128),
        )
        x1_sb = cpool.tile([128, WPC * D], f32)
        h2_sb = cpool.tile([128, WPC * D], bf16)

        ohpool = ctx.enter_context(tc.tile_pool(name="oh", bufs=32))
        pspool = ctx.enter_context(tc.tile_pool(name="ps", bufs=4, space="PSUM"))

        L1lo = CW * GL * 128
        L1hi = CW * GH * 128
        L2 = CW2 * G2 * 128
        reg1lo = nc.gpsimd.alloc_register("n_idx_1lo")
        nc.gpsimd.reg_mov(reg1lo, L1lo)
        if L1hi != L1lo:
            reg1hi = nc.gpsimd.alloc_register("n_idx_1hi")
            nc.gpsimd.reg_mov(reg1hi, L1hi)
        else:
            reg1hi = reg1lo
        reg2 = nc.gpsimd.alloc_register("n_idx_2")
        nc.gpsimd.reg_mov(reg2, L2)

        def onehot(dstl_sb, sti):
            oh = ohpool.tile([128, 128], bf16, tag="oh")
            nc.vector.tensor_scalar(
                out=oh[:],
                in0=iota_sb[:],
                scalar1=dstl_sb[:, sti:sti + 1],
                scalar2=None,
                op0=OP.is_equal,
            )
            return oh

        # ================= Round 1: dst-grouped =================
        r1ctx = ExitStack()
        gpool = r1ctx.enter_context(tc.tile_pool(name="gath1", bufs=2))
        mpool = r1ctx.enter_context(tc.tile_pool(name="misc1", bufs=4))
        opool = r1ctx.enter_context(tc.tile_pool(name="outs1", bufs=2))
        for wlo in range(0, WPC, CW):
            glow = gpool.tile([128, CW * GL, D], bf16, tag="glow")
            ghigh = gpool.tile([128, CW * GH, D], bf16, tag="ghigh")
            s0 = wlo * GL * 128
            _dma_gather_thin(
                nc.gpsimd,
                out_ap=glow[:, :, :],
                in_ap=xg[0:SPLIT, 0:D],
                idxs_ap=idx1_sb[:, s0 // 16:(s0 + L1lo) // 16],
                num_idxs=L1lo,
                num_idxs_reg=reg1lo,
                elem_size=D,
                elem_step=2 * D,
            )
            s0h = HI0 + wlo * GH * 128
            _dma_gather_thin(
                nc.gpsimd,
                out_ap=ghigh[:, :, :],
                in_ap=xg[SPLIT:NPAD, 0:D],
                idxs_ap=idx1_sb[:, s0h // 16:(s0h + L1hi) // 16],
                num_idxs=L1hi,
                num_idxs_reg=reg1hi,
                elem_size=D,
                elem_step=2 * D,
                queue_num=1,
            )
            ps = pspool.tile([128, CW * D], f32)
            for wi in range(CW):
                w = wlo + wi
                for t in range(G):
                    if t < GL:
                        g_ap = glow[:, wi * GL + t, :]
                        sti = w * GL + t
                    else:
                        g_ap = ghigh[:, wi * GH + (t - GL), :]
                        sti = WPC * GL + w * GH + (t - GL)
                    nc.tensor.matmul(
                        ps[:, wi * D:(wi + 1) * D],
                        lhsT=onehot(dstl1_sb, sti)[:],
                        rhs=g_ap,
                        start=(t == 0),
                        stop=(t == G - 1),
                    )
            # group combine: X1 = a1*dinv.h + b1*X0 ; x1g' = dinv.X1 (bf16)
            grp = slice(wlo * D, (wlo + CW) * D)
            dh = mpool.tile([128, CW * D], f32, tag="dh")
            nc.vector.tensor_tensor(
                out=dh[:], in0=ps[:], in1=dinvb_sb[:, grp], op=OP.mult
            )
            tmp = mpool.tile([128, CW * D], f32, tag="t1")
            nc.scalar.activation(tmp[:], dh[:], AF.Copy, scale=a1)
            if b1 == 1.0:
                nc.vector.tensor_tensor(
                    out=x1_sb[:, grp], in0=tmp[:], in1=x0_sb[:, grp], op=OP.add
                )
            else:
                xb = mpool.tile([128, CW * D], f32, tag="t2")
                nc.scalar.activation(xb[:], x0_sb[:, grp], AF.Copy, scale=b1)
                nc.vector.tensor_tensor(
                    out=x1_sb[:, grp], in0=tmp[:], in1=xb[:], op=OP.add
                )
            x1bf = opool.tile([128, CW * D], bf16, tag="x1bf")
            nc.vector.tensor_tensor(
                out=x1bf[:], in0=x1_sb[:, grp], in1=dinvb_sb[:, grp], op=OP.mult
            )
            nc.sync.dma_start(
                out=x1g[wlo * 128:(wlo + CW) * 128, 0:D].rearrange(
                    "(w p) d -> p w d", p=128
                ),
                in_=x1bf[:].rearrange("p (w d) -> p w d", d=D),
            )

        r1ctx.close()

        # ================= Round 2: src-grouped partials =================
        r2ctx = ExitStack()
        gpool2 = r2ctx.enter_context(tc.tile_pool(name="gath2", bufs=2))
        opool2 = r2ctx.enter_context(tc.tile_pool(name="outs2", bufs=2))
        for ci, glo in enumerate(range(0, NWIN, CW2)):
            g2 = gpool2.tile([128, CW2 * G2, D], bf16, tag="g2")
            s0 = glo * G2 * 128
            _dma_gather_thin(
                nc.gpsimd,
                out_ap=g2[:, :, :],
                in_ap=x1g[0:ROWS, 0:D],
                idxs_ap=idx2_sb[:, s0 // 16:(s0 + L2) // 16],
                num_idxs=L2,
                num_idxs_reg=reg2,
                elem_size=D,
                elem_step=2 * D,
                queue_num=ci % 2,
            )
            hst = opool2.tile([128, CW2 * D], bf16, tag="hst")
            for gi in range(0, CW2, GRP):
                ps = pspool.tile([128, GRP * D], f32)
                for wi in range(GRP):
                    gw = glo + gi + wi
                    for t in range(G2):
                        nc.tensor.matmul(
                            ps[:, wi * D:(wi + 1) * D],
                            lhsT=onehot(dstl2_sb, gw * G2 + t)[:],
                            rhs=g2[:, (gi + wi) * G2 + t, :],
                            start=(t == 0),
                            stop=(t == G2 - 1),
                        )
                nc.scalar.activation(
                    hst[:, gi * D:(gi + GRP) * D], ps[:], AF.Copy
                )
            nc.sync.dma_start(
                out=hpart[glo * 128:(glo + CW2) * 128, :].rearrange(
                    "(w p) d -> p w d", p=128
                ),
                in_=hst[:].rearrange("p (w d) -> p w d", d=D),
            )

        r2ctx.close()

        # ================= ReduceScatter =================
        nc.gpsimd.collective_compute(
            "ReduceScatter",
            mybir.AluOpType.add,
            replica_groups=[list(range(NCORES))],
            ins=[hpart[:]],
            outs=[hrs[:]],
        )

        # ================= Post: X2 + relu + output =================
        mpool = ctx.enter_context(tc.tile_pool(name="misc3", bufs=2))
        opool = ctx.enter_context(tc.tile_pool(name="outs3", bufs=2))
        nc.sync.dma_start(
            out=h2_sb[:].rearrange("p (w d) -> p w d", d=D),
            in_=hrs[:].rearrange("(w p) d -> p w d", p=128),
        )
        for gi in range(NGRP):
            grp = slice(gi * GRP * D, (gi + 1) * GRP * D)
            dh2 = mpool.tile([128, GRP * D], f32, tag="dh2")
            nc.vector.tensor_tensor(
                out=dh2[:], in0=h2_sb[:, grp], in1=dinvb_sb[:, grp], op=OP.mult
            )
            t1 = mpool.tile([128, GRP * D], f32, tag="t1b")
            nc.scalar.activation(t1[:], dh2[:], AF.Copy, scale=a2)
            x1b = mpool.tile([128, GRP * D], f32, tag="x1b")
            nc.scalar.activation(x1b[:], x1_sb[:, grp], AF.Copy, scale=b2)
            t2 = mpool.tile([128, GRP * D], f32, tag="t2b")
            nc.vector.tensor_tensor(out=t2[:], in0=t1[:], in1=x1b[:], op=OP.add)
            x2 = mpool.tile([128, GRP * D], f32, tag="x2")
            nc.vector.tensor_tensor(
                out=x2[:], in0=t2[:], in1=x0_sb[:, grp], op=OP.subtract
            )
            outg = opool.tile([128, GRP * 3 * D], f32, tag="outg")
            for wi in range(GRP):
                ob = wi * 3 * D
                ws = slice((gi * GRP + wi) * D, (gi * GRP + wi + 1) * D)
                nc.scalar.activation(outg[:, ob:ob + D], x0_sb[:, ws], AF.Relu)
                nc.scalar.activation(
                    outg[:, ob + D:ob + 2 * D], x1_sb[:, ws], AF.Relu
                )
                nc.scalar.activation(
                    outg[:, ob + 2 * D:ob + 3 * D],
                    x2[:, wi * D:(wi + 1) * D],
                    AF.Relu,
                )
            nc.sync.dma_start(
                out=out_d[gi * GRP * 128:(gi + 1) * GRP * 128, :].rearrange(
                    "(w p) d -> p w d", p=128
                ),
                in_=outg[:].rearrange("p (w d) -> p w d", d=3 * D),
            )

    nc.finalize()
    return nc


def _make_in_maps(feat, src, dst):
    import ml_dtypes

    row_of, GL, GH, G2, tables = _preprocess(src, dst)

    deg = np.bincount(np.asarray(dst, np.int64), minlength=N).astype(np.float32)
    dinv = np.maximum(deg, 1.0) ** -0.5
    dinv_pad = np.ones(NPAD, np.float32)
    dinv_pad[:N] = dinv

    xpad = np.zeros((NPAD, D), np.float32)
    xpad[:N] = np.asarray(feat, np.float32)

    # permute into storage-row order
    perm = np.empty(NPAD, np.int64)  # perm[row] = natural node
    perm[row_of] = np.arange(NPAD)
    x_rows = xpad[perm]
    dinv_rows = dinv_pad[perm]

    xg = np.zeros((NPAD, 2 * D), ml_dtypes.bfloat16)
    xg[:, :D] = (dinv_rows[:, None] * x_rows).astype(ml_dtypes.bfloat16)

    in_maps = []
    for c in range(NCORES):
        t = tables[c]
        blk = slice(c * ROWS, (c + 1) * ROWS)
        dinvb = (
            dinv_rows[blk]
            .reshape(WPC, 128)
            .T.reshape(128, WPC, 1)
            .repeat(D, axis=2)
            .reshape(128, WPC * D)
            .astype(np.float32)
        )
        in_maps.append(
            {
                "xg": xg,
                "x0own": x_rows[blk],
                "dinvb": dinvb,
                "idx1": t["idx1"],
                "dstl1": t["dstl1"],
                "idx2": t["idx2"],
                "dstl2": t["dstl2"],
            }
        )
    return row_of, GL, GH, G2, in_maps


_CACHE = {}


def _get_program(feat, src, dst, lambda_max):
    re_norm = float(
        np.float32(2.0) / np.asarray(lambda_max, np.float32).reshape(-1)[0]
    )
    key0 = (id(feat), id(src), id(dst))
    if _CACHE.get("inkey") != key0:
        _CACHE["inmaps"] = _make_in_maps(feat, src, dst)
        _CACHE["inkey"] = key0
    row_of, GL, GH, G2, in_maps = _CACHE["inmaps"]
    key = (GL, GH, G2, re_norm)
    if key not in _CACHE:
        _CACHE[key] = _build_bass(GL, GH, G2, re_norm)
    return _CACHE[key], in_maps, row_of


def kernel(feat, src, dst, lambda_max):
    from concourse.bass_utils import run_bass_kernel_spmd

    nc, in_maps, row_of = _get_program(feat, src, dst, lambda_max)
    res = run_bass_kernel_spmd(nc, in_maps, core_ids=list(range(NCORES)))
    kernel.last_exec_time_ns = res.exec_time_ns
    out_rows = np.concatenate(
        [res.results[c]["out"] for c in range(NCORES)], axis=0
    )
    return np.ascontiguousarray(out_rows[row_of[:N]])


# revision 23
# speedup vs baseline: 1.6987x; 1.1354x over previous
"""ChebConv (K=3) GNN message passing on 8 Trainium2 NeuronCores.

Strategy (v2):
  - Nodes are permuted into NPAD=61440 storage rows (8 blocks of ROWS=7680,
    60 windows of 128 per block) by a host-side greedy balancer that evens
    out per-window edge counts for both rounds (minimizes tile padding).
  - D^-1/2 normalization is folded into the gathered feature rows
    (x' = dinv*x, host-side for round 1, fused on-device for round 2) and a
    per-dst-node multiply after aggregation; one-hot lhsT tiles are pure 0/1
    indicators (padding slots get an out-of-range dst-local id -> zero col).
  - Round 1 groups edges by DST core (full x0 is resident on every core, so
    no communication): thin dma_gathers (128B payload / 256B stride) fetch
    x'[src] rows; per 6-window PSUM group, one-hot matmuls accumulate the
    segment-sum, then X1 = a1*dinv.h + b1*X0 node-locally.
  - Round 2 groups edges by SRC core (x1[src] is then core-local): each core
    computes partial h2 for ALL 480 global windows from its own X1 block,
    writes bf16 partials to DRAM, and a single ReduceScatter(add) both sums
    the partials and delivers each core its own dst block. The only
    collective is that RS (~8x smaller output than an X1 AllGather).
  - Post phase: X2 = a2*dinv.h2 + b2*X1 - X0, relu, concat, output.
"""

import sys

for _p in ("/opt/trn_rl_repo",):
    if _p not in sys.path:
        sys.path.insert(0, _p)

import numpy as np

# Problem shape (hardcoded per contract).
N, E, D = 60000, 1200000, 64
NCORES = 8
WIN = 128             # dst nodes per window (PSUM partition dim)
WPC = 60              # windows per core
ROWS = WPC * WIN      # 7680 rows per core block
NPAD = NCORES * ROWS  # 61440 padded node rows
NWIN = NCORES * WPC   # 480 global windows
SPLIT = 32768         # int16 gather index range per source half (round 1)
CW = 6                # round-1 windows per chunk (and PSUM group size)
CW2 = 24              # round-2 windows per chunk (4 PSUM groups of 6)
GRP = 6               # windows per PSUM group
PAD_DSTL = 300.0      # out-of-range dst-local id for padding slots


def _dma_gather_thin(
    gp,
    out_ap,
    in_ap,
    idxs_ap,
    num_idxs,
    num_idxs_reg,
    elem_size,
    elem_step,
    queue_num=0,
    single_packet=False,
):
    """dma_gather with payload < 256B (elem_size*dtype need not be a 256B
    multiple); the source row stride (elem_step*dtype) still must be."""
    import concourse.mybir as mybir
    from concourse import ap_utils

    gp._assert_queue_num(queue_num)
    assert idxs_ap.dtype == mybir.dt.int16
    assert in_ap.dtype == out_ap.dtype
    assert in_ap.space.name == "DRAM"
    assert idxs_ap.space.name == "SBUF"
    assert out_ap.space.name == "SBUF"
    assert ap_utils.ap_is_contiguous(out_ap.ap[1:])
    assert ap_utils.ap_is_contiguous(idxs_ap.ap[1:])
    assert in_ap.ap[-1][1] == out_ap.ap[-1][1] == elem_size
    assert out_ap.ap[0][1] * out_ap.ap[1][1] == ((num_idxs + 127) // 128) * 128
    assert in_ap.ap[0][0] == elem_step
    stride_bytes = elem_step * mybir.dt.size(in_ap.dtype)
    assert stride_bytes % 256 == 0
    stride_bytes_256 = stride_bytes // 256
    assert stride_bytes_256 < 256

    _in_ap = gp.lower_ap_dma(in_ap, for_custom_bir_dma=True)
    _idxs_ap = gp.lower_ap(idxs_ap)
    _out_ap = gp.lower_ap(out_ap)
    return gp.add_instruction(
        mybir.InstDMAGatherAnt(
            name=gp.bass.get_next_instruction_name(),
            ins=[
                *_in_ap,
                _idxs_ap,
                gp.lower_val_access(gp.to_reg(num_idxs_reg)),
            ],
            outs=[_out_ap],
            transpose=False,
            num_idxs=num_idxs,
            elem_size=elem_size,
            stride_bytes_256=stride_bytes_256,
            gen_mode=0,
            single_packet=single_packet,
            queue_num=queue_num,
            sbuf_tokens_per_rank=0,
            sbuf_free_dim_per_rank=0,
            sbuf_free_dim_pad_per_rank=0,
            sbuf_byte_offset=0,
        )
    )


def _balance_rows(src, dst):
    """Greedy node->storage-row permutation balancing per-window edge counts.

    Keeps each node in its natural core block; within a block distributes the
    7680 nodes over 60 windows to even out (a) in-edge counts split by src
    half (round-1 low/high gather streams) and (b) in-edge counts split by
    src core (round-2 partial aggregation cells).

    Returns row_of[NPAD]: natural node id -> storage row.
    """
    core_of_nat = np.minimum(np.arange(NPAD) // ROWS, NCORES - 1)
    src_core = np.minimum(src // ROWS, NCORES - 1)

    # provisional half classification by natural id (exact except for core-4
    # srcs whose final row may cross SPLIT; core 4 is balanced first so its
    # final rows are used for every other block)
    row_of = np.arange(NPAD, dtype=np.int64)

    def node_features(c, row_of):
        nodes = np.arange(c * ROWS, (c + 1) * ROWS, dtype=np.int64)
        # in-edges by src core: M[i, j] = #edges (src in core j) -> node i
        sel = np.minimum(dst // ROWS, NCORES - 1) == c
        d_loc = dst[sel] - c * ROWS
        F = np.zeros((ROWS, 10), np.int64)
        np.add.at(F, (d_loc, src_core[sel]), 1)
        low = row_of[src[sel]] < SPLIT
        np.add.at(F, (d_loc[low], 8), 1)
        np.add.at(F, (d_loc[~low], 9), 1)
        return nodes, F

    order_c = [4] + [c for c in range(NCORES) if c != 4]
    for c in order_c:
        nodes, F = node_features(c, row_of)
        tot = F.sum(axis=0).astype(np.float64)
        target = np.maximum(tot / WPC, 1.0)
        # process nodes by total degree, heaviest first
        deg_tot = F[:, :8].sum(axis=1)
        nd_order = np.argsort(-deg_tot, kind="stable")
        W = np.zeros((WPC, 10), np.float64)
        nfill = np.zeros(WPC, np.int64)
        win_of = np.zeros(ROWS, np.int64)
        Fn = F.astype(np.float64)
        for i in nd_order:
            score = ((W + Fn[i]) / target).max(axis=1)
            score[nfill >= WIN] = np.inf
            w = int(np.argmin(score))
            win_of[i] = w
            W[w] += Fn[i]
            nfill[w] += 1
        # rows: window-major, slot order by node id within window
        idx_sorted = np.lexsort((np.arange(ROWS), win_of))
        rows = c * ROWS + np.arange(ROWS)
        row_of[nodes[idx_sorted]] = rows
    return row_of


def _preprocess(src, dst):
    """Build balanced permutation + per-core gather/one-hot tables."""
    src = np.asarray(src, dtype=np.int64)
    dst = np.asarray(dst, dtype=np.int64)

    row_of = _balance_rows(src, dst)
    srcr = row_of[src]
    dstr = row_of[dst]

    gwin = dstr // WIN
    dcore = gwin // WPC
    wloc = gwin % WPC
    scoreid = srcr // ROWS
    low = srcr < SPLIT
    dstl = (dstr % WIN).astype(np.float32)

    # ---- Round 1 (dst-grouped) ----
    nlo = np.zeros((NCORES, WPC), np.int64)
    nhi = np.zeros((NCORES, WPC), np.int64)
    np.add.at(nlo, (dcore[low], wloc[low]), 1)
    np.add.at(nhi, (dcore[~low], wloc[~low]), 1)
    GL = int(np.max((nlo + 127) // 128))
    GH = int(np.max((nhi + 127) // 128))
    T1 = WPC * (GL + GH)
    S1 = T1 * 128
    HI0 = WPC * GL * 128

    key1 = (dcore * WPC + wloc) * 2 + (~low)
    order1 = np.argsort(key1 * (1 << 17) + srcr, kind="stable")
    s1_src = srcr[order1]
    s1_dstl = dstl[order1]
    counts1 = np.zeros(NCORES * WPC * 2, np.int64)
    np.add.at(counts1, key1, 1)
    starts1 = np.concatenate([[0], np.cumsum(counts1)])

    # ---- Round 2 (src-grouped over all 480 global windows) ----
    cnt2 = np.zeros((NCORES, NWIN), np.int64)
    np.add.at(cnt2, (scoreid, gwin), 1)
    G2 = int(np.max((cnt2 + 127) // 128))
    T2 = NWIN * G2
    S2 = T2 * 128

    key2 = scoreid * NWIN + gwin
    order2 = np.argsort(key2 * (1 << 13) + (srcr % ROWS), kind="stable")
    s2_src = (srcr % ROWS)[order2]
    s2_dstl = dstl[order2]
    counts2 = np.zeros(NCORES * NWIN, np.int64)
    np.add.at(counts2, key2, 1)
    starts2 = np.concatenate([[0], np.cumsum(counts2)])

    tables = []
    for c in range(NCORES):
        idx1 = np.zeros(S1, np.int16)
        dstl1 = np.full(S1, PAD_DSTL, np.float32)
        for w in range(WPC):
            kbase = (c * WPC + w) * 2
            a, b = starts1[kbase], starts1[kbase + 1]
            n = b - a
            o = w * GL * 128
            if n:
                idx1[o:o + n] = s1_src[a:b].astype(np.int16)
                dstl1[o:o + n] = s1_dstl[a:b]
            a, b = starts1[kbase + 1], starts1[kbase + 2]
            n = b - a
            o = HI0 + w * GH * 128
            if n:
                idx1[o:o + n] = (s1_src[a:b] - SPLIT).astype(np.int16)
                dstl1[o:o + n] = s1_dstl[a:b]

        idx2 = np.zeros(S2, np.int16)
        dstl2 = np.full(S2, PAD_DSTL, np.float32)
        for g in range(NWIN):
            k = c * NWIN + g
            a, b = starts2[k], starts2[k + 1]
            n = b - a
            o = g * G2 * 128
            if n:
                idx2[o:o + n] = s2_src[a:b].astype(np.int16)
                dstl2[o:o + n] = s2_dstl[a:b]

        tables.append(
            {
                "idx1": np.tile(idx1.reshape(S1 // 16, 16).T, (8, 1)),
                "dstl1": dstl1.reshape(T1, 128).T.copy(),
                "idx2": np.tile(idx2.reshape(S2 // 16, 16).T, (8, 1)),
                "dstl2": dstl2.reshape(T2, 128).T.copy(),
            }
        )
    return row_of, GL, GH, G2, tables


def _build_bass(GL, GH, G2, re_norm):
    import concourse.bass as bass
    import concourse.bacc as bacc
    import concourse.mybir as mybir
    import concourse.tile as tile
    import ml_dtypes
    from contextlib import ExitStack

    f32 = mybir.dt.float32
    i16 = mybir.dt.int16
    bf16 = mybir.dt.bfloat16
    AF = mybir.ActivationFunctionType
    OP = mybir.AluOpType

    G = GL + GH
    T1 = WPC * G
    S1 = T1 * 128
    HI0 = WPC * GL * 128
    T2 = NWIN * G2
    S2 = T2 * 128
    NGRP = WPC // GRP          # per-core window groups (10)
    OHT = max(G, GRP * G2)     # one-hot tiles per group allocation

    a1 = float(-re_norm)       # X1 = a1*dinv.h1 + b1*X0
    b1 = float(re_norm - 1.0)
    a2 = float(-2.0 * re_norm)  # X2 = a2*dinv.h2 + b2*X1 - X0
    b2 = float(2.0 * (re_norm - 1.0))

    nc = bacc.Bacc(
        "TRN2",
        target_bir_lowering=False,
        debug=False,
        enable_asserts=False,
        num_devices=NCORES,
        num_swdge_queues=2,
    )
    xg = nc.dram_tensor("xg", [NPAD, 2 * D], bf16, kind="ExternalInput")
    x0own = nc.dram_tensor("x0own", [ROWS, D], f32, kind="ExternalInput")
    dinvb_d = nc.dram_tensor("dinvb", [128, WPC * D], f32, kind="ExternalInput")
    a2dinvb_d = nc.dram_tensor(
        "a2dinvb", [128, WPC * D], f32, kind="ExternalInput"
    )
    idx1_d = nc.dram_tensor("idx1", [128, S1 // 16], i16, kind="ExternalInput")
    dstl1_d = nc.dram_tensor("dstl1", [128, T1], f32, kind="ExternalInput")
    idx2_d = nc.dram_tensor("idx2", [128, S2 // 16], i16, kind="ExternalInput")
    dstl2_d = nc.dram_tensor("dstl2", [128, T2], f32, kind="ExternalInput")
    out_d = nc.dram_tensor("out", [ROWS, 3 * D], f32, kind="ExternalOutput")
    iota_d = nc.inline_tensor(
        np.broadcast_to(np.arange(128), (128, 128)).astype(ml_dtypes.bfloat16),
        name="iota",
    )

    with ExitStack() as ctx:
        tc = ctx.enter_context(tile.TileContext(nc))
        dram = ctx.enter_context(tc.tile_pool(name="dram", bufs=1, space="DRAM"))
        x1g = dram.tile([ROWS, 2 * D], bf16)
        hpart = dram.tile([NPAD, D], bf16)
        hrs = dram.tile([ROWS, D], bf16)

        cpool = ctx.enter_context(tc.tile_pool(name="const", bufs=1))
        # critical-path loads first (gate the first gather / first one-hots)
        idx1_sb = cpool.tile([128, S1 // 16], i16)
        nc.sync.dma_start(out=idx1_sb[:], in_=idx1_d[:])
        dstl1_sb = cpool.tile([128, T1], f32)
        iota_sb = cpool.tile([128, 128], bf16)
        nc.scalar.dma_start(out=iota_sb[:], in_=iota_d[:])
        nc.scalar.dma_start(out=dstl1_sb[:], in_=dstl1_d[:])
        # the rest on other queues / later
        idx2_sb = cpool.tile([128, S2 // 16], i16)
        nc.sync.dma_start(out=idx2_sb[:], in_=idx2_d[:])
        dstl2_sb = cpool.tile([128, T2], f32)
        nc.scalar.dma_start(out=dstl2_sb[:], in_=dstl2_d[:])
        dinvb_sb = cpool.tile([128, WPC * D], f32)
        nc.scalar.dma_start(out=dinvb_sb[:], in_=dinvb_d[:])
        a2dinvb_sb = cpool.tile([128, WPC * D], f32)
        nc.scalar.dma_start(out=a2dinvb_sb[:], in_=a2dinvb_d[:])
        xc_sb = cpool.tile([128, WPC * D], f32)
        x0_sb = cpool.tile([128, WPC * D], f32)
        nc.scalar.dma_start(
            out=x0_sb[:].rearrange("p (w d) -> p w d", d=D),
            in_=x0own[:].rearrange("(w p) d -> p w d", p=128),
        )
        x1_sb = cpool.tile([128, WPC * D], f32)
        h2_sb = cpool.tile([128, WPC * D], bf16)

        ohpool = ctx.enter_context(tc.tile_pool(name="oh", bufs=3))
        pspool = ctx.enter_context(tc.tile_pool(name="ps", bufs=4, space="PSUM"))

        L1lo = CW * GL * 128
        L1hi = CW * GH * 128
        L2 = CW2 * G2 * 128
        reg1lo = nc.gpsimd.alloc_register("n_idx_1lo")
        nc.gpsimd.reg_mov(reg1lo, L1lo)
        if L1hi != L1lo:
            reg1hi = nc.gpsimd.alloc_register("n_idx_1hi")
            nc.gpsimd.reg_mov(reg1hi, L1hi)
        else:
            reg1hi = reg1lo
        reg2 = nc.gpsimd.alloc_register("n_idx_2")
        nc.gpsimd.reg_mov(reg2, L2)

        def onehot(ohg, ti, dstl_sb, sti):
            nc.vector.tensor_scalar(
                out=ohg[:, ti, :],
                in0=iota_sb[:],
                scalar1=dstl_sb[:, sti:sti + 1],
                scalar2=None,
                op0=OP.is_equal,
            )
            return ohg[:, ti, :]

        # ================= Round 1: dst-grouped =================
        r1ctx = ExitStack()
        gpool = r1ctx.enter_context(tc.tile_pool(name="gath1", bufs=2))
        mpool = r1ctx.enter_context(tc.tile_pool(name="misc1", bufs=4))
        opool = r1ctx.enter_context(tc.tile_pool(name="outs1", bufs=2))
        for wlo in range(0, WPC, CW):
            glow = gpool.tile([128, CW * GL, D], bf16, tag="glow")
            ghigh = gpool.tile([128, CW * GH, D], bf16, tag="ghigh")
            s0 = wlo * GL * 128
            _dma_gather_thin(
                nc.gpsimd,
                out_ap=glow[:, :, :],
                in_ap=xg[0:SPLIT, 0:D],
                idxs_ap=idx1_sb[:, s0 // 16:(s0 + L1lo) // 16],
                num_idxs=L1lo,
                num_idxs_reg=reg1lo,
                elem_size=D,
                elem_step=2 * D,
            )
            s0h = HI0 + wlo * GH * 128
            _dma_gather_thin(
                nc.gpsimd,
                out_ap=ghigh[:, :, :],
                in_ap=xg[SPLIT:NPAD, 0:D],
                idxs_ap=idx1_sb[:, s0h // 16:(s0h + L1hi) // 16],
                num_idxs=L1hi,
                num_idxs_reg=reg1hi,
                elem_size=D,
                elem_step=2 * D,
                queue_num=1,
            )
            ps = pspool.tile([128, CW * D], f32)
            for wi in range(CW):
                w = wlo + wi
                ohg = ohpool.tile([128, OHT, 128], bf16, tag="ohg")
                for t in range(G):
                    if t < GL:
                        g_ap = glow[:, wi * GL + t, :]
                        sti = w * GL + t
                    else:
                        g_ap = ghigh[:, wi * GH + (t - GL), :]
                        sti = WPC * GL + w * GH + (t - GL)
                    nc.tensor.matmul(
                        ps[:, wi * D:(wi + 1) * D],
                        lhsT=onehot(ohg, t, dstl1_sb, sti),
                        rhs=g_ap,
                        start=(t == 0),
                        stop=(t == G - 1),
                    )
            # group combine: X1 = a1*dinv.h + b1*X0 ; x1g' = dinv.X1 (bf16)
            grp = slice(wlo * D, (wlo + CW) * D)
            dh = mpool.tile([128, CW * D], f32, tag="dh")
            nc.vector.tensor_tensor(
                out=dh[:], in0=ps[:], in1=dinvb_sb[:, grp], op=OP.mult
            )
            tmp = mpool.tile([128, CW * D], f32, tag="t1")
            nc.scalar.activation(tmp[:], dh[:], AF.Copy, scale=a1)
            if b1 == 1.0:
                nc.vector.tensor_tensor(
                    out=x1_sb[:, grp], in0=tmp[:], in1=x0_sb[:, grp], op=OP.add
                )
            else:
                xb = mpool.tile([128, CW * D], f32, tag="t2")
                nc.scalar.activation(xb[:], x0_sb[:, grp], AF.Copy, scale=b1)
                nc.vector.tensor_tensor(
                    out=x1_sb[:, grp], in0=tmp[:], in1=xb[:], op=OP.add
                )
            x1bf = opool.tile([128, CW * D], bf16, tag="x1bf")
            nc.vector.tensor_tensor(
                out=x1bf[:], in0=x1_sb[:, grp], in1=dinvb_sb[:, grp], op=OP.mult
            )
            nc.sync.dma_start(
                out=x1g[wlo * 128:(wlo + CW) * 128, 0:D].rearrange(
                    "(w p) d -> p w d", p=128
                ),
                in_=x1bf[:].rearrange("p (w d) -> p w d", d=D),
            )
            # pre-compute xc = b2*X1 - X0 for the post phase
            xb2 = mpool.tile([128, CW * D], f32, tag="xb2")
            nc.scalar.activation(xb2[:], x1_sb[:, grp], AF.Copy, scale=b2)
            nc.vector.tensor_tensor(
                out=xc_sb[:, grp], in0=xb2[:], in1=x0_sb[:, grp], op=OP.subtract
            )

        r1ctx.close()

        # ================= Round 2: src-grouped partials =================
        r2ctx = ExitStack()
        gpool2 = r2ctx.enter_context(tc.tile_pool(name="gath2", bufs=2))
        opool2 = r2ctx.enter_context(tc.tile_pool(name="outs2", bufs=2))
        for ci, glo in enumerate(range(0, NWIN, CW2)):
            g2 = gpool2.tile([128, CW2 * G2, D], bf16, tag="g2")
            s0 = glo * G2 * 128
            _dma_gather_thin(
                nc.gpsimd,
                out_ap=g2[:, :, :],
                in_ap=x1g[0:ROWS, 0:D],
                idxs_ap=idx2_sb[:, s0 // 16:(s0 + L2) // 16],
                num_idxs=L2,
                num_idxs_reg=reg2,
                elem_size=D,
                elem_step=2 * D,
                queue_num=ci % 2,
            )
            hst = opool2.tile([128, CW2 * D], bf16, tag="hst")
            for gi in range(0, CW2, GRP):
                ps = pspool.tile([128, GRP * D], f32)
                ohg = ohpool.tile([128, OHT, 128], bf16, tag="ohg")
                for wi in range(GRP):
                    gw = glo + gi + wi
                    for t in range(G2):
                        nc.tensor.matmul(
                            ps[:, wi * D:(wi + 1) * D],
                            lhsT=onehot(ohg, wi * G2 + t, dstl2_sb, gw * G2 + t),
                            rhs=g2[:, (gi + wi) * G2 + t, :],
                            start=(t == 0),
                            stop=(t == G2 - 1),
                        )
                nc.scalar.activation(
                    hst[:, gi * D:(gi + GRP) * D], ps[:], AF.Copy
                )
            nc.sync.dma_start(
                out=hpart[glo * 128:(glo + CW2) * 128, :].rearrange(
                    "(w p) d -> p w d", p=128
                ),
                in_=hst[:].rearrange("p (w d) -> p w d", d=D),
            )

        r2ctx.close()

        # ================= ReduceScatter =================
        nc.gpsimd.collective_compute(
            "ReduceScatter",
            mybir.AluOpType.add,
            replica_groups=[list(range(NCORES))],
            ins=[hpart[:]],
            outs=[hrs[:]],
        )

        # ================= Post: X2 + relu + output =================
        mpool = ctx.enter_context(tc.tile_pool(name="misc3", bufs=3))
        opool = ctx.enter_context(tc.tile_pool(name="outs3", bufs=3))
        nc.sync.dma_start(
            out=h2_sb[:].rearrange("p (w d) -> p w d", d=D),
            in_=hrs[:].rearrange("(w p) d -> p w d", p=128),
        )
        for gi in range(NGRP):
            grp = slice(gi * GRP * D, (gi + 1) * GRP * D)
            t1 = mpool.tile([128, GRP * D], f32, tag="t1b")
            nc.vector.tensor_tensor(
                out=t1[:], in0=h2_sb[:, grp], in1=a2dinvb_sb[:, grp], op=OP.mult
            )
            x2 = mpool.tile([128, GRP * D], f32, tag="x2")
            nc.vector.tensor_tensor(
                out=x2[:], in0=t1[:], in1=xc_sb[:, grp], op=OP.add
            )
            outg = opool.tile([128, GRP * 3 * D], f32, tag="outg")
            for wi in range(GRP):
                ob = wi * 3 * D
                ws = slice((gi * GRP + wi) * D, (gi * GRP + wi + 1) * D)
                nc.scalar.activation(outg[:, ob:ob + D], x0_sb[:, ws], AF.Relu)
                nc.scalar.activation(
                    outg[:, ob + D:ob + 2 * D], x1_sb[:, ws], AF.Relu
                )
                nc.scalar.activation(
                    outg[:, ob + 2 * D:ob + 3 * D],
                    x2[:, wi * D:(wi + 1) * D],
                    AF.Relu,
                )
            nc.sync.dma_start(
                out=out_d[gi * GRP * 128:(gi + 1) * GRP * 128, :].rearrange(
                    "(w p) d -> p w d", p=128
                ),
                in_=outg[:].rearrange("p (w d) -> p w d", d=3 * D),
            )

    nc.finalize()
    return nc


def _make_in_maps(feat, src, dst, re_norm):
    import ml_dtypes

    row_of, GL, GH, G2, tables = _preprocess(src, dst)

    deg = np.bincount(np.asarray(dst, np.int64), minlength=N).astype(np.float32)
    dinv = np.maximum(deg, 1.0) ** -0.5
    dinv_pad = np.ones(NPAD, np.float32)
    dinv_pad[:N] = dinv

    xpad = np.zeros((NPAD, D), np.float32)
    xpad[:N] = np.asarray(feat, np.float32)

    # permute into storage-row order
    perm = np.empty(NPAD, np.int64)  # perm[row] = natural node
    perm[row_of] = np.arange(NPAD)
    x_rows = xpad[perm]
    dinv_rows = dinv_pad[perm]

    xg = np.zeros((NPAD, 2 * D), ml_dtypes.bfloat16)
    xg[:, :D] = (dinv_rows[:, None] * x_rows).astype(ml_dtypes.bfloat16)

    in_maps = []
    for c in range(NCORES):
        t = tables[c]
        blk = slice(c * ROWS, (c + 1) * ROWS)
        dinvb = (
            dinv_rows[blk]
            .reshape(WPC, 128)
            .T.reshape(128, WPC, 1)
            .repeat(D, axis=2)
            .reshape(128, WPC * D)
            .astype(np.float32)
        )
        in_maps.append(
            {
                "xg": xg,
                "x0own": x_rows[blk],
                "dinvb": dinvb,
                "a2dinvb": (-2.0 * re_norm) * dinvb,
                "idx1": t["idx1"],
                "dstl1": t["dstl1"],
                "idx2": t["idx2"],
                "dstl2": t["dstl2"],
            }
        )
    return row_of, GL, GH, G2, in_maps


_CACHE = {}


def _get_program(feat, src, dst, lambda_max):
    re_norm = float(
        np.float32(2.0) / np.asarray(lambda_max, np.float32).reshape(-1)[0]
    )
    key0 = (id(feat), id(src), id(dst), re_norm)
    if _CACHE.get("inkey") != key0:
        _CACHE["inmaps"] = _make_in_maps(feat, src, dst, re_norm)
        _CACHE["inkey"] = key0
    row_of, GL, GH, G2, in_maps = _CACHE["inmaps"]
    key = (GL, GH, G2, re_norm)
    if key not in _CACHE:
        _CACHE[key] = _build_bass(GL, GH, G2, re_norm)
    return _CACHE[key], in_maps, row_of


def kernel(feat, src, dst, lambda_max):
    from concourse.bass_utils import run_bass_kernel_spmd

    nc, in_maps, row_of = _get_program(feat, src, dst, lambda_max)
    res = run_bass_kernel_spmd(nc, in_maps, core_ids=list(range(NCORES)))
    kernel.last_exec_time_ns = res.exec_time_ns
    out_rows = np.concatenate(
        [res.results[c]["out"] for c in range(NCORES)], axis=0
    )
    return np.ascontiguousarray(out_rows[row_of[:N]])


# revision 25
# speedup vs baseline: 1.7039x; 1.0031x over previous
"""ChebConv (K=3) GNN message passing on 8 Trainium2 NeuronCores.

Strategy (v2):
  - Nodes are permuted into NPAD=61440 storage rows (8 blocks of ROWS=7680,
    60 windows of 128 per block) by a host-side greedy balancer that evens
    out per-window edge counts for both rounds (minimizes tile padding).
  - D^-1/2 normalization is folded into the gathered feature rows
    (x' = dinv*x, host-side for round 1, fused on-device for round 2) and a
    per-dst-node multiply after aggregation; one-hot lhsT tiles are pure 0/1
    indicators (padding slots get an out-of-range dst-local id -> zero col).
  - Round 1 groups edges by DST core (full x0 is resident on every core, so
    no communication): thin dma_gathers (128B payload / 256B stride) fetch
    x'[src] rows; per 6-window PSUM group, one-hot matmuls accumulate the
    segment-sum, then X1 = a1*dinv.h + b1*X0 node-locally.
  - Round 2 groups edges by SRC core (x1[src] is then core-local): each core
    computes partial h2 for ALL 480 global windows from its own X1 block,
    writes bf16 partials to DRAM, and a single ReduceScatter(add) both sums
    the partials and delivers each core its own dst block. The only
    collective is that RS (~8x smaller output than an X1 AllGather).
  - Post phase: X2 = a2*dinv.h2 + b2*X1 - X0, relu, concat, output.
"""

import sys

for _p in ("/opt/trn_rl_repo",):
    if _p not in sys.path:
        sys.path.insert(0, _p)

import numpy as np

# Problem shape (hardcoded per contract).
N, E, D = 60000, 1200000, 64
NCORES = 8
WIN = 128             # dst nodes per window (PSUM partition dim)
WPC = 60              # windows per core
ROWS = WPC * WIN      # 7680 rows per core block
NPAD = NCORES * ROWS  # 61440 padded node rows
NWIN = NCORES * WPC   # 480 global windows
SPLIT = 32768         # int16 gather index range per source half (round 1)
CW = 6                # round-1 windows per chunk (and PSUM group size)
CW2 = 24              # round-2 windows per chunk (4 PSUM groups of 6)
GRP = 6               # windows per PSUM group
PAD_DSTL = 300.0      # out-of-range dst-local id for padding slots


def _dma_gather_thin(
    gp,
    out_ap,
    in_ap,
    idxs_ap,
    num_idxs,
    num_idxs_reg,
    elem_size,
    elem_step,
    queue_num=0,
    single_packet=False,
):
    """dma_gather with payload < 256B (elem_size*dtype need not be a 256B
    multiple); the source row stride (elem_step*dtype) still must be."""
    import concourse.mybir as mybir
    from concourse import ap_utils

    gp._assert_queue_num(queue_num)
    assert idxs_ap.dtype == mybir.dt.int16
    assert in_ap.dtype == out_ap.dtype
    assert in_ap.space.name == "DRAM"
    assert idxs_ap.space.name == "SBUF"
    assert out_ap.space.name == "SBUF"
    assert ap_utils.ap_is_contiguous(out_ap.ap[1:])
    assert ap_utils.ap_is_contiguous(idxs_ap.ap[1:])
    assert in_ap.ap[-1][1] == out_ap.ap[-1][1] == elem_size
    assert out_ap.ap[0][1] * out_ap.ap[1][1] == ((num_idxs + 127) // 128) * 128
    assert in_ap.ap[0][0] == elem_step
    stride_bytes = elem_step * mybir.dt.size(in_ap.dtype)
    assert stride_bytes % 256 == 0
    stride_bytes_256 = stride_bytes // 256
    assert stride_bytes_256 < 256

    _in_ap = gp.lower_ap_dma(in_ap, for_custom_bir_dma=True)
    _idxs_ap = gp.lower_ap(idxs_ap)
    _out_ap = gp.lower_ap(out_ap)
    return gp.add_instruction(
        mybir.InstDMAGatherAnt(
            name=gp.bass.get_next_instruction_name(),
            ins=[
                *_in_ap,
                _idxs_ap,
                gp.lower_val_access(gp.to_reg(num_idxs_reg)),
            ],
            outs=[_out_ap],
            transpose=False,
            num_idxs=num_idxs,
            elem_size=elem_size,
            stride_bytes_256=stride_bytes_256,
            gen_mode=0,
            single_packet=single_packet,
            queue_num=queue_num,
            sbuf_tokens_per_rank=0,
            sbuf_free_dim_per_rank=0,
            sbuf_free_dim_pad_per_rank=0,
            sbuf_byte_offset=0,
        )
    )


def _balance_rows(src, dst):
    """Greedy node->storage-row permutation balancing per-window edge counts.

    Keeps each node in its natural core block; within a block distributes the
    7680 nodes over 60 windows to even out (a) in-edge counts split by src
    half (round-1 low/high gather streams) and (b) in-edge counts split by
    src core (round-2 partial aggregation cells).

    Returns row_of[NPAD]: natural node id -> storage row.
    """
    core_of_nat = np.minimum(np.arange(NPAD) // ROWS, NCORES - 1)
    src_core = np.minimum(src // ROWS, NCORES - 1)

    # provisional half classification by natural id (exact except for core-4
    # srcs whose final row may cross SPLIT; core 4 is balanced first so its
    # final rows are used for every other block)
    row_of = np.arange(NPAD, dtype=np.int64)

    def node_features(c, row_of):
        nodes = np.arange(c * ROWS, (c + 1) * ROWS, dtype=np.int64)
        # in-edges by src core: M[i, j] = #edges (src in core j) -> node i
        sel = np.minimum(dst // ROWS, NCORES - 1) == c
        d_loc = dst[sel] - c * ROWS
        F = np.zeros((ROWS, 10), np.int64)
        np.add.at(F, (d_loc, src_core[sel]), 1)
        low = row_of[src[sel]] < SPLIT
        np.add.at(F, (d_loc[low], 8), 1)
        np.add.at(F, (d_loc[~low], 9), 1)
        return nodes, F

    order_c = [4] + [c for c in range(NCORES) if c != 4]
    for c in order_c:
        nodes, F = node_features(c, row_of)
        tot = F.sum(axis=0).astype(np.float64)
        target = np.maximum(tot / WPC, 1.0)
        # process nodes by total degree, heaviest first
        deg_tot = F[:, :8].sum(axis=1)
        nd_order = np.argsort(-deg_tot, kind="stable")
        W = np.zeros((WPC, 10), np.float64)
        nfill = np.zeros(WPC, np.int64)
        win_of = np.zeros(ROWS, np.int64)
        Fn = F.astype(np.float64)
        for i in nd_order:
            score = ((W + Fn[i]) / target).max(axis=1)
            score[nfill >= WIN] = np.inf
            w = int(np.argmin(score))
            win_of[i] = w
            W[w] += Fn[i]
            nfill[w] += 1
        # rows: window-major, slot order by node id within window
        idx_sorted = np.lexsort((np.arange(ROWS), win_of))
        rows = c * ROWS + np.arange(ROWS)
        row_of[nodes[idx_sorted]] = rows
    return row_of


def _preprocess(src, dst):
    """Build balanced permutation + per-core gather/one-hot tables."""
    src = np.asarray(src, dtype=np.int64)
    dst = np.asarray(dst, dtype=np.int64)

    row_of = _balance_rows(src, dst)
    srcr = row_of[src]
    dstr = row_of[dst]

    gwin = dstr // WIN
    dcore = gwin // WPC
    wloc = gwin % WPC
    scoreid = srcr // ROWS
    low = srcr < SPLIT
    dstl = (dstr % WIN).astype(np.float32)

    # ---- Round 1 (dst-grouped) ----
    nlo = np.zeros((NCORES, WPC), np.int64)
    nhi = np.zeros((NCORES, WPC), np.int64)
    np.add.at(nlo, (dcore[low], wloc[low]), 1)
    np.add.at(nhi, (dcore[~low], wloc[~low]), 1)
    GL = int(np.max((nlo + 127) // 128))
    GH = int(np.max((nhi + 127) // 128))
    T1 = WPC * (GL + GH)
    S1 = T1 * 128
    HI0 = WPC * GL * 128

    key1 = (dcore * WPC + wloc) * 2 + (~low)
    order1 = np.argsort(key1 * (1 << 17) + srcr, kind="stable")
    s1_src = srcr[order1]
    s1_dstl = dstl[order1]
    counts1 = np.zeros(NCORES * WPC * 2, np.int64)
    np.add.at(counts1, key1, 1)
    starts1 = np.concatenate([[0], np.cumsum(counts1)])

    # ---- Round 2 (src-grouped over all 480 global windows) ----
    cnt2 = np.zeros((NCORES, NWIN), np.int64)
    np.add.at(cnt2, (scoreid, gwin), 1)
    G2 = int(np.max((cnt2 + 127) // 128))
    T2 = NWIN * G2
    S2 = T2 * 128

    key2 = scoreid * NWIN + gwin
    order2 = np.argsort(key2 * (1 << 13) + (srcr % ROWS), kind="stable")
    s2_src = (srcr % ROWS)[order2]
    s2_dstl = dstl[order2]
    counts2 = np.zeros(NCORES * NWIN, np.int64)
    np.add.at(counts2, key2, 1)
    starts2 = np.concatenate([[0], np.cumsum(counts2)])

    tables = []
    for c in range(NCORES):
        idx1 = np.zeros(S1, np.int16)
        dstl1 = np.full(S1, PAD_DSTL, np.float32)
        for w in range(WPC):
            kbase = (c * WPC + w) * 2
            a, b = starts1[kbase], starts1[kbase + 1]
            n = b - a
            o = w * GL * 128
            if n:
                idx1[o:o + n] = s1_src[a:b].astype(np.int16)
                dstl1[o:o + n] = s1_dstl[a:b]
            a, b = starts1[kbase + 1], starts1[kbase + 2]
            n = b - a
            o = HI0 + w * GH * 128
            if n:
                idx1[o:o + n] = (s1_src[a:b] - SPLIT).astype(np.int16)
                dstl1[o:o + n] = s1_dstl[a:b]

        idx2 = np.zeros(S2, np.int16)
        dstl2 = np.full(S2, PAD_DSTL, np.float32)
        for g in range(NWIN):
            k = c * NWIN + g
            a, b = starts2[k], starts2[k + 1]
            n = b - a
            o = g * G2 * 128
            if n:
                idx2[o:o + n] = s2_src[a:b].astype(np.int16)
                dstl2[o:o + n] = s2_dstl[a:b]

        tables.append(
            {
                "idx1": np.tile(idx1.reshape(S1 // 16, 16).T, (8, 1)),
                "dstl1": dstl1.reshape(T1, 128).T.copy(),
                "idx2": np.tile(idx2.reshape(S2 // 16, 16).T, (8, 1)),
                "dstl2": dstl2.reshape(T2, 128).T.copy(),
            }
        )
    return row_of, GL, GH, G2, tables


def _build_bass(GL, GH, G2, re_norm):
    import concourse.bass as bass
    import concourse.bacc as bacc
    import concourse.mybir as mybir
    import concourse.tile as tile
    import ml_dtypes
    from contextlib import ExitStack

    f32 = mybir.dt.float32
    i16 = mybir.dt.int16
    bf16 = mybir.dt.bfloat16
    AF = mybir.ActivationFunctionType
    OP = mybir.AluOpType

    G = GL + GH
    T1 = WPC * G
    S1 = T1 * 128
    HI0 = WPC * GL * 128
    T2 = NWIN * G2
    S2 = T2 * 128
    NGRP = WPC // GRP          # per-core window groups (10)
    OHT = max(G, GRP * G2)     # one-hot tiles per group allocation

    a1 = float(-re_norm)       # X1 = a1*dinv.h1 + b1*X0
    b1 = float(re_norm - 1.0)
    a2 = float(-2.0 * re_norm)  # X2 = a2*dinv.h2 + b2*X1 - X0
    b2 = float(2.0 * (re_norm - 1.0))

    nc = bacc.Bacc(
        "TRN2",
        target_bir_lowering=False,
        debug=False,
        enable_asserts=False,
        num_devices=NCORES,
        num_swdge_queues=2,
    )
    xg = nc.dram_tensor("xg", [NPAD, 2 * D], bf16, kind="ExternalInput")
    x0own = nc.dram_tensor("x0own", [ROWS, D], f32, kind="ExternalInput")
    dinvb_d = nc.dram_tensor("dinvb", [128, WPC * D], f32, kind="ExternalInput")
    a2dinvb_d = nc.dram_tensor(
        "a2dinvb", [128, WPC * D], f32, kind="ExternalInput"
    )
    idx1_d = nc.dram_tensor("idx1", [128, S1 // 16], i16, kind="ExternalInput")
    dstl1_d = nc.dram_tensor("dstl1", [128, T1], f32, kind="ExternalInput")
    idx2_d = nc.dram_tensor("idx2", [128, S2 // 16], i16, kind="ExternalInput")
    dstl2_d = nc.dram_tensor("dstl2", [128, T2], f32, kind="ExternalInput")
    out_d = nc.dram_tensor("out", [ROWS, 3 * D], f32, kind="ExternalOutput")
    iota_d = nc.inline_tensor(
        np.broadcast_to(np.arange(128), (128, 128)).astype(ml_dtypes.bfloat16),
        name="iota",
    )

    with ExitStack() as ctx:
        tc = ctx.enter_context(tile.TileContext(nc))
        dram = ctx.enter_context(tc.tile_pool(name="dram", bufs=1, space="DRAM"))
        x1g = dram.tile([ROWS, 2 * D], bf16)
        hpart = dram.tile([NPAD, D], bf16)
        hrs = dram.tile([ROWS, D], bf16)

        cpool = ctx.enter_context(tc.tile_pool(name="const", bufs=1))
        # critical-path loads first (gate the first gather / first one-hots)
        idx1_sb = cpool.tile([128, S1 // 16], i16)
        nc.sync.dma_start(out=idx1_sb[:], in_=idx1_d[:])
        dstl1_sb = cpool.tile([128, T1], f32)
        iota_sb = cpool.tile([128, 128], bf16)
        nc.scalar.dma_start(out=iota_sb[:], in_=iota_d[:])
        nc.scalar.dma_start(out=dstl1_sb[:], in_=dstl1_d[:])
        # the rest on other queues / later
        idx2_sb = cpool.tile([128, S2 // 16], i16)
        nc.sync.dma_start(out=idx2_sb[:], in_=idx2_d[:])
        dstl2_sb = cpool.tile([128, T2], f32)
        nc.scalar.dma_start(out=dstl2_sb[:], in_=dstl2_d[:])
        dinvb_sb = cpool.tile([128, WPC * D], f32)
        nc.scalar.dma_start(out=dinvb_sb[:], in_=dinvb_d[:])
        a2dinvb_sb = cpool.tile([128, WPC * D], f32)
        nc.scalar.dma_start(out=a2dinvb_sb[:], in_=a2dinvb_d[:])
        xc_sb = cpool.tile([128, WPC * D], f32)
        x0_sb = cpool.tile([128, WPC * D], f32)
        nc.scalar.dma_start(
            out=x0_sb[:].rearrange("p (w d) -> p w d", d=D),
            in_=x0own[:].rearrange("(w p) d -> p w d", p=128),
        )
        x1_sb = cpool.tile([128, WPC * D], f32)
        h2_sb = cpool.tile([128, WPC * D], bf16)

        ohpool = ctx.enter_context(tc.tile_pool(name="oh", bufs=4))
        pspool = ctx.enter_context(tc.tile_pool(name="ps", bufs=6, space="PSUM"))

        L1lo = CW * GL * 128
        L1hi = CW * GH * 128
        L2 = CW2 * G2 * 128
        reg1lo = nc.gpsimd.alloc_register("n_idx_1lo")
        nc.gpsimd.reg_mov(reg1lo, L1lo)
        if L1hi != L1lo:
            reg1hi = nc.gpsimd.alloc_register("n_idx_1hi")
            nc.gpsimd.reg_mov(reg1hi, L1hi)
        else:
            reg1hi = reg1lo
        reg2 = nc.gpsimd.alloc_register("n_idx_2")
        nc.gpsimd.reg_mov(reg2, L2)

        def onehot(ohg, ti, dstl_sb, sti):
            nc.vector.tensor_scalar(
                out=ohg[:, ti, :],
                in0=iota_sb[:],
                scalar1=dstl_sb[:, sti:sti + 1],
                scalar2=None,
                op0=OP.is_equal,
            )
            return ohg[:, ti, :]

        # ================= Round 1: dst-grouped =================
        r1ctx = ExitStack()
        gpool = r1ctx.enter_context(tc.tile_pool(name="gath1", bufs=2))
        mpool = r1ctx.enter_context(tc.tile_pool(name="misc1", bufs=2))
        opool = r1ctx.enter_context(tc.tile_pool(name="outs1", bufs=2))
        for wlo in range(0, WPC, CW):
            glow = gpool.tile([128, CW * GL, D], bf16, tag="glow")
            ghigh = gpool.tile([128, CW * GH, D], bf16, tag="ghigh")
            s0 = wlo * GL * 128
            _dma_gather_thin(
                nc.gpsimd,
                out_ap=glow[:, :, :],
                in_ap=xg[0:SPLIT, 0:D],
                idxs_ap=idx1_sb[:, s0 // 16:(s0 + L1lo) // 16],
                num_idxs=L1lo,
                num_idxs_reg=reg1lo,
                elem_size=D,
                elem_step=2 * D,
            )
            s0h = HI0 + wlo * GH * 128
            _dma_gather_thin(
                nc.gpsimd,
                out_ap=ghigh[:, :, :],
                in_ap=xg[SPLIT:NPAD, 0:D],
                idxs_ap=idx1_sb[:, s0h // 16:(s0h + L1hi) // 16],
                num_idxs=L1hi,
                num_idxs_reg=reg1hi,
                elem_size=D,
                elem_step=2 * D,
                queue_num=1,
            )
            ps = pspool.tile([128, CW * D], f32)
            for wi in range(CW):
                w = wlo + wi
                ohg = ohpool.tile([128, OHT, 128], bf16, tag="ohg")
                for t in range(G):
                    if t < GL:
                        g_ap = glow[:, wi * GL + t, :]
                        sti = w * GL + t
                    else:
                        g_ap = ghigh[:, wi * GH + (t - GL), :]
                        sti = WPC * GL + w * GH + (t - GL)
                    nc.tensor.matmul(
                        ps[:, wi * D:(wi + 1) * D],
                        lhsT=onehot(ohg, t, dstl1_sb, sti),
                        rhs=g_ap,
                        start=(t == 0),
                        stop=(t == G - 1),
                    )
            # group combine: X1 = a1*dinv.h + b1*X0 ; x1g' = dinv.X1 (bf16)
            grp = slice(wlo * D, (wlo + CW) * D)
            dh = mpool.tile([128, CW * D], f32, tag="dh")
            nc.vector.tensor_tensor(
                out=dh[:], in0=ps[:], in1=dinvb_sb[:, grp], op=OP.mult
            )
            tmp = mpool.tile([128, CW * D], f32, tag="t1")
            nc.scalar.activation(tmp[:], dh[:], AF.Copy, scale=a1)
            if b1 == 1.0:
                nc.vector.tensor_tensor(
                    out=x1_sb[:, grp], in0=tmp[:], in1=x0_sb[:, grp], op=OP.add
                )
            else:
                xb = mpool.tile([128, CW * D], f32, tag="t2")
                nc.scalar.activation(xb[:], x0_sb[:, grp], AF.Copy, scale=b1)
                nc.vector.tensor_tensor(
                    out=x1_sb[:, grp], in0=tmp[:], in1=xb[:], op=OP.add
                )
            x1bf = opool.tile([128, CW * D], bf16, tag="x1bf")
            nc.vector.tensor_tensor(
                out=x1bf[:], in0=x1_sb[:, grp], in1=dinvb_sb[:, grp], op=OP.mult
            )
            nc.sync.dma_start(
                out=x1g[wlo * 128:(wlo + CW) * 128, 0:D].rearrange(
                    "(w p) d -> p w d", p=128
                ),
                in_=x1bf[:].rearrange("p (w d) -> p w d", d=D),
            )
            # pre-compute xc = b2*X1 - X0 for the post phase
            xb2 = mpool.tile([128, CW * D], f32, tag="xb2")
            nc.scalar.activation(xb2[:], x1_sb[:, grp], AF.Copy, scale=b2)
            nc.vector.tensor_tensor(
                out=xc_sb[:, grp], in0=xb2[:], in1=x0_sb[:, grp], op=OP.subtract
            )

        r1ctx.close()

        # ================= Round 2: src-grouped partials =================
        r2ctx = ExitStack()
        gpool2 = r2ctx.enter_context(tc.tile_pool(name="gath2", bufs=2))
        opool2 = r2ctx.enter_context(tc.tile_pool(name="outs2", bufs=2))
        for ci, glo in enumerate(range(0, NWIN, CW2)):
            g2 = gpool2.tile([128, CW2 * G2, D], bf16, tag="g2")
            s0 = glo * G2 * 128
            _dma_gather_thin(
                nc.gpsimd,
                out_ap=g2[:, :, :],
                in_ap=x1g[0:ROWS, 0:D],
                idxs_ap=idx2_sb[:, s0 // 16:(s0 + L2) // 16],
                num_idxs=L2,
                num_idxs_reg=reg2,
                elem_size=D,
                elem_step=2 * D,
                queue_num=ci % 2,
            )
            hst = opool2.tile([128, CW2 * D], bf16, tag="hst")
            for gi in range(0, CW2, GRP):
                ps = pspool.tile([128, GRP * D], f32)
                ohg = ohpool.tile([128, OHT, 128], bf16, tag="ohg")
                for wi in range(GRP):
                    gw = glo + gi + wi
                    for t in range(G2):
                        nc.tensor.matmul(
                            ps[:, wi * D:(wi + 1) * D],
                            lhsT=onehot(ohg, wi * G2 + t, dstl2_sb, gw * G2 + t),
                            rhs=g2[:, (gi + wi) * G2 + t, :],
                            start=(t == 0),
                            stop=(t == G2 - 1),
                        )
                nc.scalar.activation(
                    hst[:, gi * D:(gi + GRP) * D], ps[:], AF.Copy
                )
            nc.sync.dma_start(
                out=hpart[glo * 128:(glo + CW2) * 128, :].rearrange(
                    "(w p) d -> p w d", p=128
                ),
                in_=hst[:].rearrange("p (w d) -> p w d", d=D),
            )

        r2ctx.close()

        # ================= ReduceScatter =================
        nc.gpsimd.collective_compute(
            "ReduceScatter",
            mybir.AluOpType.add,
            replica_groups=[list(range(NCORES))],
            ins=[hpart[:]],
            outs=[hrs[:]],
        )

        # ================= Post: X2 + relu + output =================
        mpool = ctx.enter_context(tc.tile_pool(name="misc3", bufs=3))
        opool = ctx.enter_context(tc.tile_pool(name="outs3", bufs=3))
        nc.sync.dma_start(
            out=h2_sb[:].rearrange("p (w d) -> p w d", d=D),
            in_=hrs[:].rearrange("(w p) d -> p w d", p=128),
        )
        for gi in range(NGRP):
            grp = slice(gi * GRP * D, (gi + 1) * GRP * D)
            t1 = mpool.tile([128, GRP * D], f32, tag="t1b")
            nc.vector.tensor_tensor(
                out=t1[:], in0=h2_sb[:, grp], in1=a2dinvb_sb[:, grp], op=OP.mult
            )
            x2 = mpool.tile([128, GRP * D], f32, tag="x2")
            nc.vector.tensor_tensor(
                out=x2[:], in0=t1[:], in1=xc_sb[:, grp], op=OP.add
            )
            outg = opool.tile([128, GRP * 3 * D], f32, tag="outg")
            for wi in range(GRP):
                ob = wi * 3 * D
                ws = slice((gi * GRP + wi) * D, (gi * GRP + wi + 1) * D)
                nc.scalar.activation(outg[:, ob:ob + D], x0_sb[:, ws], AF.Relu)
                nc.scalar.activation(
                    outg[:, ob + D:ob + 2 * D], x1_sb[:, ws], AF.Relu
                )
                nc.scalar.activation(
                    outg[:, ob + 2 * D:ob + 3 * D],
                    x2[:, wi * D:(wi + 1) * D],
                    AF.Relu,
                )
            nc.sync.dma_start(
                out=out_d[gi * GRP * 128:(gi + 1) * GRP * 128, :].rearrange(
                    "(w p) d -> p w d", p=128
                ),
                in_=outg[:].rearrange("p (w d) -> p w d", d=3 * D),
            )

    nc.finalize()
    return nc


def _make_in_maps(feat, src, dst, re_norm):
    import ml_dtypes

    row_of, GL, GH, G2, tables = _preprocess(src, dst)

    deg = np.bincount(np.asarray(dst, np.int64), minlength=N).astype(np.float32)
    dinv = np.maximum(deg, 1.0) ** -0.5
    dinv_pad = np.ones(NPAD, np.float32)
    dinv_pad[:N] = dinv

    xpad = np.zeros((NPAD, D), np.float32)
    xpad[:N] = np.asarray(feat, np.float32)

    # permute into storage-row order
    perm = np.empty(NPAD, np.int64)  # perm[row] = natural node
    perm[row_of] = np.arange(NPAD)
    x_rows = xpad[perm]
    dinv_rows = dinv_pad[perm]

    xg = np.zeros((NPAD, 2 * D), ml_dtypes.bfloat16)
    xg[:, :D] = (dinv_rows[:, None] * x_rows).astype(ml_dtypes.bfloat16)

    in_maps = []
    for c in range(NCORES):
        t = tables[c]
        blk = slice(c * ROWS, (c + 1) * ROWS)
        dinvb = (
            dinv_rows[blk]
            .reshape(WPC, 128)
            .T.reshape(128, WPC, 1)
            .repeat(D, axis=2)
            .reshape(128, WPC * D)
            .astype(np.float32)
        )
        in_maps.append(
            {
                "xg": xg,
                "x0own": x_rows[blk],
                "dinvb": dinvb,
                "a2dinvb": (-2.0 * re_norm) * dinvb,
                "idx1": t["idx1"],
                "dstl1": t["dstl1"],
                "idx2": t["idx2"],
                "dstl2": t["dstl2"],
            }
        )
    return row_of, GL, GH, G2, in_maps


_CACHE = {}


def _get_program(feat, src, dst, lambda_max):
    re_norm = float(
        np.float32(2.0) / np.asarray(lambda_max, np.float32).reshape(-1)[0]
    )
    key0 = (id(feat), id(src), id(dst), re_norm)
    if _CACHE.get("inkey") != key0:
        _CACHE["inmaps"] = _make_in_maps(feat, src, dst, re_norm)
        _CACHE["inkey"] = key0
    row_of, GL, GH, G2, in_maps = _CACHE["inmaps"]
    key = (GL, GH, G2, re_norm)
    if key not in _CACHE:
        _CACHE[key] = _build_bass(GL, GH, G2, re_norm)
    return _CACHE[key], in_maps, row_of


def kernel(feat, src, dst, lambda_max):
    from concourse.bass_utils import run_bass_kernel_spmd

    nc, in_maps, row_of = _get_program(feat, src, dst, lambda_max)
    res = run_bass_kernel_spmd(nc, in_maps, core_ids=list(range(NCORES)))
    kernel.last_exec_time_ns = res.exec_time_ns
    out_rows = np.concatenate(
        [res.results[c]["out"] for c in range(NCORES)], axis=0
    )
    return np.ascontiguousarray(out_rows[row_of[:N]])


# revision 27
# speedup vs baseline: 1.7249x; 1.0123x over previous
"""ChebConv (K=3) GNN message passing on 8 Trainium2 NeuronCores.

Strategy (v2):
  - Nodes are permuted into NPAD=61440 storage rows (8 blocks of ROWS=7680,
    60 windows of 128 per block) by a host-side greedy balancer that evens
    out per-window edge counts for both rounds (minimizes tile padding).
  - D^-1/2 normalization is folded into the gathered feature rows
    (x' = dinv*x, host-side for round 1, fused on-device for round 2) and a
    per-dst-node multiply after aggregation; one-hot lhsT tiles are pure 0/1
    indicators (padding slots get an out-of-range dst-local id -> zero col).
  - Round 1 groups edges by DST core (full x0 is resident on every core, so
    no communication): thin dma_gathers (128B payload / 256B stride) fetch
    x'[src] rows; per 6-window PSUM group, one-hot matmuls accumulate the
    segment-sum, then X1 = a1*dinv.h + b1*X0 node-locally.
  - Round 2 groups edges by SRC core (x1[src] is then core-local): each core
    computes partial h2 for ALL 480 global windows from its own X1 block,
    writes bf16 partials to DRAM, and a single ReduceScatter(add) both sums
    the partials and delivers each core its own dst block. The only
    collective is that RS (~8x smaller output than an X1 AllGather).
  - Post phase: X2 = a2*dinv.h2 + b2*X1 - X0, relu, concat, output.
"""

import sys

for _p in ("/opt/trn_rl_repo",):
    if _p not in sys.path:
        sys.path.insert(0, _p)

import numpy as np

# Problem shape (hardcoded per contract).
N, E, D = 60000, 1200000, 64
NCORES = 8
WIN = 128             # dst nodes per window (PSUM partition dim)
WPC = 60              # windows per core
ROWS = WPC * WIN      # 7680 rows per core block
NPAD = NCORES * ROWS  # 61440 padded node rows
NWIN = NCORES * WPC   # 480 global windows
SPLIT = 32768         # int16 gather index range per source half (round 1)
CW = 6                # round-1 windows per chunk (and PSUM group size)
CW2 = 24              # round-2 windows per chunk (4 PSUM groups of 6)
GRP = 6               # windows per PSUM group
PAD_DSTL = 300.0      # out-of-range dst-local id for padding slots


def _dma_gather_thin(
    gp,
    out_ap,
    in_ap,
    idxs_ap,
    num_idxs,
    num_idxs_reg,
    elem_size,
    elem_step,
    queue_num=0,
    single_packet=False,
):
    """dma_gather with payload < 256B (elem_size*dtype need not be a 256B
    multiple); the source row stride (elem_step*dtype) still must be."""
    import concourse.mybir as mybir
    from concourse import ap_utils

    gp._assert_queue_num(queue_num)
    assert idxs_ap.dtype == mybir.dt.int16
    assert in_ap.dtype == out_ap.dtype
    assert in_ap.space.name == "DRAM"
    assert idxs_ap.space.name == "SBUF"
    assert out_ap.space.name == "SBUF"
    assert ap_utils.ap_is_contiguous(out_ap.ap[1:])
    assert ap_utils.ap_is_contiguous(idxs_ap.ap[1:])
    assert in_ap.ap[-1][1] == out_ap.ap[-1][1] == elem_size
    assert out_ap.ap[0][1] * out_ap.ap[1][1] == ((num_idxs + 127) // 128) * 128
    assert in_ap.ap[0][0] == elem_step
    stride_bytes = elem_step * mybir.dt.size(in_ap.dtype)
    assert stride_bytes % 256 == 0
    stride_bytes_256 = stride_bytes // 256
    assert stride_bytes_256 < 256

    _in_ap = gp.lower_ap_dma(in_ap, for_custom_bir_dma=True)
    _idxs_ap = gp.lower_ap(idxs_ap)
    _out_ap = gp.lower_ap(out_ap)
    return gp.add_instruction(
        mybir.InstDMAGatherAnt(
            name=gp.bass.get_next_instruction_name(),
            ins=[
                *_in_ap,
                _idxs_ap,
                gp.lower_val_access(gp.to_reg(num_idxs_reg)),
            ],
            outs=[_out_ap],
            transpose=False,
            num_idxs=num_idxs,
            elem_size=elem_size,
            stride_bytes_256=stride_bytes_256,
            gen_mode=0,
            single_packet=single_packet,
            queue_num=queue_num,
            sbuf_tokens_per_rank=0,
            sbuf_free_dim_per_rank=0,
            sbuf_free_dim_pad_per_rank=0,
            sbuf_byte_offset=0,
        )
    )


def _balance_rows(src, dst):
    """Greedy node->storage-row permutation balancing per-window edge counts.

    Keeps each node in its natural core block; within a block distributes the
    7680 nodes over 60 windows to even out (a) in-edge counts split by src
    half (round-1 low/high gather streams) and (b) in-edge counts split by
    src core (round-2 partial aggregation cells).

    Returns row_of[NPAD]: natural node id -> storage row.
    """
    core_of_nat = np.minimum(np.arange(NPAD) // ROWS, NCORES - 1)
    src_core = np.minimum(src // ROWS, NCORES - 1)

    # provisional half classification by natural id (exact except for core-4
    # srcs whose final row may cross SPLIT; core 4 is balanced first so its
    # final rows are used for every other block)
    row_of = np.arange(NPAD, dtype=np.int64)

    def node_features(c, row_of):
        nodes = np.arange(c * ROWS, (c + 1) * ROWS, dtype=np.int64)
        # in-edges by src core: M[i, j] = #edges (src in core j) -> node i
        sel = np.minimum(dst // ROWS, NCORES - 1) == c
        d_loc = dst[sel] - c * ROWS
        F = np.zeros((ROWS, 10), np.int64)
        np.add.at(F, (d_loc, src_core[sel]), 1)
        low = row_of[src[sel]] < SPLIT
        np.add.at(F, (d_loc[low], 8), 1)
        np.add.at(F, (d_loc[~low], 9), 1)
        return nodes, F

    order_c = [4] + [c for c in range(NCORES) if c != 4]
    for c in order_c:
        nodes, F = node_features(c, row_of)
        tot = F.sum(axis=0).astype(np.float64)
        target = np.maximum(tot / WPC, 1.0)
        # process nodes by total degree, heaviest first
        deg_tot = F[:, :8].sum(axis=1)
        nd_order = np.argsort(-deg_tot, kind="stable")
        W = np.zeros((WPC, 10), np.float64)
        nfill = np.zeros(WPC, np.int64)
        win_of = np.zeros(ROWS, np.int64)
        Fn = F.astype(np.float64)
        for i in nd_order:
            score = ((W + Fn[i]) / target).max(axis=1)
            score[nfill >= WIN] = np.inf
            w = int(np.argmin(score))
            win_of[i] = w
            W[w] += Fn[i]
            nfill[w] += 1
        # rows: window-major, slot order by node id within window
        idx_sorted = np.lexsort((np.arange(ROWS), win_of))
        rows = c * ROWS + np.arange(ROWS)
        row_of[nodes[idx_sorted]] = rows
    return row_of


def _preprocess(src, dst):
    """Build balanced permutation + per-core gather/one-hot tables."""
    import ml_dtypes
    global _bf16np
    _bf16np = ml_dtypes.bfloat16
    src = np.asarray(src, dtype=np.int64)
    dst = np.asarray(dst, dtype=np.int64)

    row_of = _balance_rows(src, dst)
    srcr = row_of[src]
    dstr = row_of[dst]

    gwin = dstr // WIN
    dcore = gwin // WPC
    wloc = gwin % WPC
    scoreid = srcr // ROWS
    low = srcr < SPLIT
    dstl = (dstr % WIN).astype(np.float32)

    # ---- Round 1 (dst-grouped) ----
    nlo = np.zeros((NCORES, WPC), np.int64)
    nhi = np.zeros((NCORES, WPC), np.int64)
    np.add.at(nlo, (dcore[low], wloc[low]), 1)
    np.add.at(nhi, (dcore[~low], wloc[~low]), 1)
    GL = int(np.max((nlo + 127) // 128))
    GH = int(np.max((nhi + 127) // 128))
    T1 = WPC * (GL + GH)
    S1 = T1 * 128
    HI0 = WPC * GL * 128

    key1 = (dcore * WPC + wloc) * 2 + (~low)
    order1 = np.argsort(key1 * (1 << 17) + srcr, kind="stable")
    s1_src = srcr[order1]
    s1_dstl = dstl[order1]
    counts1 = np.zeros(NCORES * WPC * 2, np.int64)
    np.add.at(counts1, key1, 1)
    starts1 = np.concatenate([[0], np.cumsum(counts1)])

    # ---- Round 2 (src-grouped over all 480 global windows) ----
    cnt2 = np.zeros((NCORES, NWIN), np.int64)
    np.add.at(cnt2, (scoreid, gwin), 1)
    G2 = int(np.max((cnt2 + 127) // 128))
    T2 = NWIN * G2
    S2 = T2 * 128

    key2 = scoreid * NWIN + gwin
    order2 = np.argsort(key2 * (1 << 13) + (srcr % ROWS), kind="stable")
    s2_src = (srcr % ROWS)[order2]
    s2_dstl = dstl[order2]
    counts2 = np.zeros(NCORES * NWIN, np.int64)
    np.add.at(counts2, key2, 1)
    starts2 = np.concatenate([[0], np.cumsum(counts2)])

    tables = []
    for c in range(NCORES):
        idx1 = np.zeros(S1, np.int16)
        dstl1 = np.full(S1, PAD_DSTL, np.float32)
        for w in range(WPC):
            kbase = (c * WPC + w) * 2
            a, b = starts1[kbase], starts1[kbase + 1]
            n = b - a
            o = w * GL * 128
            if n:
                idx1[o:o + n] = s1_src[a:b].astype(np.int16)
                dstl1[o:o + n] = s1_dstl[a:b]
            a, b = starts1[kbase + 1], starts1[kbase + 2]
            n = b - a
            o = HI0 + w * GH * 128
            if n:
                idx1[o:o + n] = (s1_src[a:b] - SPLIT).astype(np.int16)
                dstl1[o:o + n] = s1_dstl[a:b]

        idx2 = np.zeros(S2, np.int16)
        dstl2 = np.full(S2, PAD_DSTL, np.float32)
        for g in range(NWIN):
            k = c * NWIN + g
            a, b = starts2[k], starts2[k + 1]
            n = b - a
            o = g * G2 * 128
            if n:
                idx2[o:o + n] = s2_src[a:b].astype(np.int16)
                dstl2[o:o + n] = s2_dstl[a:b]

        tables.append(
            {
                "idx1": np.tile(idx1.reshape(S1 // 16, 16).T, (8, 1)),
                "dstl1": dstl1.reshape(T1, 128).T.copy(),
                "idx2": np.tile(idx2.reshape(S2 // 16, 16).T, (8, 1)),
                "dstl2": dstl2.reshape(T2, 128).T.copy(),
            }
        )
    return row_of, GL, GH, G2, tables


def _build_bass(GL, GH, G2, re_norm):
    import concourse.bass as bass
    import concourse.bacc as bacc
    import concourse.mybir as mybir
    import concourse.tile as tile
    import ml_dtypes
    from contextlib import ExitStack

    f32 = mybir.dt.float32
    i16 = mybir.dt.int16
    bf16 = mybir.dt.bfloat16
    AF = mybir.ActivationFunctionType
    OP = mybir.AluOpType

    G = GL + GH
    T1 = WPC * G
    S1 = T1 * 128
    HI0 = WPC * GL * 128
    T2 = NWIN * G2
    S2 = T2 * 128
    NGRP = WPC // GRP          # per-core window groups (10)
    OHT = max(G, GRP * G2)     # one-hot tiles per group allocation

    a1 = float(-re_norm)       # X1 = a1*dinv.h1 + b1*X0
    b1 = float(re_norm - 1.0)
    a2 = float(-2.0 * re_norm)  # X2 = a2*dinv.h2 + b2*X1 - X0
    b2 = float(2.0 * (re_norm - 1.0))

    nc = bacc.Bacc(
        "TRN2",
        target_bir_lowering=False,
        debug=False,
        enable_asserts=False,
        num_devices=NCORES,
        num_swdge_queues=2,
    )
    xg = nc.dram_tensor("xg", [NPAD, 2 * D], bf16, kind="ExternalInput")
    x0own = nc.dram_tensor("x0own", [ROWS, D], f32, kind="ExternalInput")
    dinvb_d = nc.dram_tensor("dinvb", [128, WPC * D], f32, kind="ExternalInput")
    idx1_d = nc.dram_tensor("idx1", [128, S1 // 16], i16, kind="ExternalInput")
    dstl1_d = nc.dram_tensor("dstl1", [128, T1], f32, kind="ExternalInput")
    idx2_d = nc.dram_tensor("idx2", [128, S2 // 16], i16, kind="ExternalInput")
    dstl2_d = nc.dram_tensor("dstl2", [128, T2], f32, kind="ExternalInput")
    out_d = nc.dram_tensor("out", [ROWS, 3 * D], f32, kind="ExternalOutput")
    iota_d = nc.inline_tensor(
        np.broadcast_to(np.arange(128), (128, 128)).astype(ml_dtypes.bfloat16),
        name="iota",
    )

    with ExitStack() as ctx:
        tc = ctx.enter_context(tile.TileContext(nc))
        dram = ctx.enter_context(tc.tile_pool(name="dram", bufs=1, space="DRAM"))
        x1g = dram.tile([ROWS, 2 * D], bf16)
        hpart = dram.tile([NPAD, D], bf16)
        hrs = dram.tile([ROWS, D], bf16)

        cpool = ctx.enter_context(tc.tile_pool(name="const", bufs=1))
        # critical-path loads first (gate the first gather / first one-hots)
        idx1_sb = cpool.tile([128, S1 // 16], i16)
        nc.sync.dma_start(out=idx1_sb[:], in_=idx1_d[:])
        dstl1_sb = cpool.tile([128, T1], f32)
        iota_sb = cpool.tile([128, 128], bf16)
        nc.scalar.dma_start(out=iota_sb[:], in_=iota_d[:])
        nc.scalar.dma_start(out=dstl1_sb[:], in_=dstl1_d[:])
        # the rest on other queues / later
        idx2_sb = cpool.tile([128, S2 // 16], i16)
        nc.sync.dma_start(out=idx2_sb[:], in_=idx2_d[:])
        dstl2_sb = cpool.tile([128, T2], f32)
        nc.scalar.dma_start(out=dstl2_sb[:], in_=dstl2_d[:])
        dinvb_sb = cpool.tile([128, WPC * D], f32)
        nc.scalar.dma_start(out=dinvb_sb[:], in_=dinvb_d[:])
        a2dinvb_sb = cpool.tile([128, WPC * D], f32)
        nc.vector.tensor_scalar(
            out=a2dinvb_sb[:], in0=dinvb_sb[:], scalar1=a2, scalar2=None,
            op0=OP.mult,
        )
        xc_sb = cpool.tile([128, WPC * D], f32)
        x0_sb = cpool.tile([128, WPC * D], f32)
        nc.scalar.dma_start(
            out=x0_sb[:].rearrange("p (w d) -> p w d", d=D),
            in_=x0own[:].rearrange("(w p) d -> p w d", p=128),
        )
        x1_sb = cpool.tile([128, WPC * D], f32)
        h2_sb = cpool.tile([128, WPC * D], bf16)

        ohpool = ctx.enter_context(tc.tile_pool(name="oh", bufs=4))
        pspool = ctx.enter_context(tc.tile_pool(name="ps", bufs=6, space="PSUM"))

        L1lo = CW * GL * 128
        L1hi = CW * GH * 128
        L2 = CW2 * G2 * 128
        reg1lo = nc.gpsimd.alloc_register("n_idx_1lo")
        nc.gpsimd.reg_mov(reg1lo, L1lo)
        if L1hi != L1lo:
            reg1hi = nc.gpsimd.alloc_register("n_idx_1hi")
            nc.gpsimd.reg_mov(reg1hi, L1hi)
        else:
            reg1hi = reg1lo
        reg2 = nc.gpsimd.alloc_register("n_idx_2")
        nc.gpsimd.reg_mov(reg2, L2)

        def onehot(ohg, ti, dstl_sb, sti):
            nc.vector.tensor_scalar(
                out=ohg[:, ti, :],
                in0=iota_sb[:],
                scalar1=dstl_sb[:, sti:sti + 1],
                scalar2=None,
                op0=OP.is_equal,
            )
            return ohg[:, ti, :]

        # ================= Round 1: dst-grouped =================
        r1ctx = ExitStack()
        gpool = r1ctx.enter_context(tc.tile_pool(name="gath1", bufs=2))
        mpool = r1ctx.enter_context(tc.tile_pool(name="misc1", bufs=2))
        opool = r1ctx.enter_context(tc.tile_pool(name="outs1", bufs=2))
        for wlo in range(0, WPC, CW):
            glow = gpool.tile([128, CW * GL, D], bf16, tag="glow")
            ghigh = gpool.tile([128, CW * GH, D], bf16, tag="ghigh")
            s0 = wlo * GL * 128
            _dma_gather_thin(
                nc.gpsimd,
                out_ap=glow[:, :, :],
                in_ap=xg[0:SPLIT, 0:D],
                idxs_ap=idx1_sb[:, s0 // 16:(s0 + L1lo) // 16],
                num_idxs=L1lo,
                num_idxs_reg=reg1lo,
                elem_size=D,
                elem_step=2 * D,
            )
            s0h = HI0 + wlo * GH * 128
            _dma_gather_thin(
                nc.gpsimd,
                out_ap=ghigh[:, :, :],
                in_ap=xg[SPLIT:NPAD, 0:D],
                idxs_ap=idx1_sb[:, s0h // 16:(s0h + L1hi) // 16],
                num_idxs=L1hi,
                num_idxs_reg=reg1hi,
                elem_size=D,
                elem_step=2 * D,
                queue_num=1,
            )
            ps = pspool.tile([128, CW * D], f32)
            for wi in range(CW):
                w = wlo + wi
                ohg = ohpool.tile([128, OHT, 128], bf16, tag="ohg")
                for t in range(G):
                    if t < GL:
                        g_ap = glow[:, wi * GL + t, :]
                        sti = w * GL + t
                    else:
                        g_ap = ghigh[:, wi * GH + (t - GL), :]
                        sti = WPC * GL + w * GH + (t - GL)
                    nc.tensor.matmul(
                        ps[:, wi * D:(wi + 1) * D],
                        lhsT=onehot(ohg, t, dstl1_sb, sti),
                        rhs=g_ap,
                        start=(t == 0),
                        stop=(t == G - 1),
                    )
            # group combine: X1 = a1*dinv.h + b1*X0 ; x1g' = dinv.X1 (bf16)
            grp = slice(wlo * D, (wlo + CW) * D)
            dh = mpool.tile([128, CW * D], f32, tag="dh")
            nc.vector.tensor_tensor(
                out=dh[:], in0=ps[:], in1=dinvb_sb[:, grp], op=OP.mult
            )
            tmp = mpool.tile([128, CW * D], f32, tag="t1")
            nc.scalar.activation(tmp[:], dh[:], AF.Copy, scale=a1)
            if b1 == 1.0:
                nc.vector.tensor_tensor(
                    out=x1_sb[:, grp], in0=tmp[:], in1=x0_sb[:, grp], op=OP.add
                )
            else:
                xb = mpool.tile([128, CW * D], f32, tag="t2")
                nc.scalar.activation(xb[:], x0_sb[:, grp], AF.Copy, scale=b1)
                nc.vector.tensor_tensor(
                    out=x1_sb[:, grp], in0=tmp[:], in1=xb[:], op=OP.add
                )
            x1bf = opool.tile([128, CW * D], bf16, tag="x1bf")
            nc.vector.tensor_tensor(
                out=x1bf[:], in0=x1_sb[:, grp], in1=dinvb_sb[:, grp], op=OP.mult
            )
            nc.sync.dma_start(
                out=x1g[wlo * 128:(wlo + CW) * 128, 0:D].rearrange(
                    "(w p) d -> p w d", p=128
                ),
                in_=x1bf[:].rearrange("p (w d) -> p w d", d=D),
            )
            # pre-compute xc = b2*X1 - X0 for the post phase
            xb2 = mpool.tile([128, CW * D], f32, tag="xb2")
            nc.scalar.activation(xb2[:], x1_sb[:, grp], AF.Copy, scale=b2)
            nc.vector.tensor_tensor(
                out=xc_sb[:, grp], in0=xb2[:], in1=x0_sb[:, grp], op=OP.subtract
            )

        r1ctx.close()

        # ================= Round 2: src-grouped partials =================
        r2ctx = ExitStack()
        gpool2 = r2ctx.enter_context(tc.tile_pool(name="gath2", bufs=2))
        opool2 = r2ctx.enter_context(tc.tile_pool(name="outs2", bufs=2))
        for ci, glo in enumerate(range(0, NWIN, CW2)):
            g2 = gpool2.tile([128, CW2 * G2, D], bf16, tag="g2")
            s0 = glo * G2 * 128
            _dma_gather_thin(
                nc.gpsimd,
                out_ap=g2[:, :, :],
                in_ap=x1g[0:ROWS, 0:D],
                idxs_ap=idx2_sb[:, s0 // 16:(s0 + L2) // 16],
                num_idxs=L2,
                num_idxs_reg=reg2,
                elem_size=D,
                elem_step=2 * D,
                queue_num=ci % 2,
            )
            hst = opool2.tile([128, CW2 * D], bf16, tag="hst")
            for gi in range(0, CW2, GRP):
                ps = pspool.tile([128, GRP * D], f32)
                ohg = ohpool.tile([128, OHT, 128], bf16, tag="ohg")
                for wi in range(GRP):
                    gw = glo + gi + wi
                    for t in range(G2):
                        nc.tensor.matmul(
                            ps[:, wi * D:(wi + 1) * D],
                            lhsT=onehot(ohg, wi * G2 + t, dstl2_sb, gw * G2 + t),
                            rhs=g2[:, (gi + wi) * G2 + t, :],
                            start=(t == 0),
                            stop=(t == G2 - 1),
                        )
                nc.scalar.activation(
                    hst[:, gi * D:(gi + GRP) * D], ps[:], AF.Copy
                )
            nc.sync.dma_start(
                out=hpart[glo * 128:(glo + CW2) * 128, :].rearrange(
                    "(w p) d -> p w d", p=128
                ),
                in_=hst[:].rearrange("p (w d) -> p w d", d=D),
            )

        r2ctx.close()

        # ================= ReduceScatter =================
        nc.gpsimd.collective_compute(
            "ReduceScatter",
            mybir.AluOpType.add,
            replica_groups=[list(range(NCORES))],
            ins=[hpart[:]],
            outs=[hrs[:]],
        )

        # ================= Post: X2 + relu + output =================
        mpool = ctx.enter_context(tc.tile_pool(name="misc3", bufs=3))
        opool = ctx.enter_context(tc.tile_pool(name="outs3", bufs=3))
        nc.sync.dma_start(
            out=h2_sb[:].rearrange("p (w d) -> p w d", d=D),
            in_=hrs[:].rearrange("(w p) d -> p w d", p=128),
        )
        for gi in range(NGRP):
            grp = slice(gi * GRP * D, (gi + 1) * GRP * D)
            t1 = mpool.tile([128, GRP * D], f32, tag="t1b")
            nc.vector.tensor_tensor(
                out=t1[:], in0=h2_sb[:, grp], in1=a2dinvb_sb[:, grp], op=OP.mult
            )
            x2 = mpool.tile([128, GRP * D], f32, tag="x2")
            nc.vector.tensor_tensor(
                out=x2[:], in0=t1[:], in1=xc_sb[:, grp], op=OP.add
            )
            outg = opool.tile([128, GRP * 3 * D], f32, tag="outg")
            for wi in range(GRP):
                ob = wi * 3 * D
                ws = slice((gi * GRP + wi) * D, (gi * GRP + wi + 1) * D)
                nc.scalar.activation(outg[:, ob:ob + D], x0_sb[:, ws], AF.Relu)
                nc.scalar.activation(
                    outg[:, ob + D:ob + 2 * D], x1_sb[:, ws], AF.Relu
                )
                nc.scalar.activation(
                    outg[:, ob + 2 * D:ob + 3 * D],
                    x2[:, wi * D:(wi + 1) * D],
                    AF.Relu,
                )
            nc.sync.dma_start(
                out=out_d[gi * GRP * 128:(gi + 1) * GRP * 128, :].rearrange(
                    "(w p) d -> p w d", p=128
                ),
                in_=outg[:].rearrange("p (w d) -> p w d", d=3 * D),
            )

    nc.finalize()
    return nc


def _make_in_maps(feat, src, dst, re_norm):
    import ml_dtypes

    row_of, GL, GH, G2, tables = _preprocess(src, dst)

    deg = np.bincount(np.asarray(dst, np.int64), minlength=N).astype(np.float32)
    dinv = np.maximum(deg, 1.0) ** -0.5
    dinv_pad = np.ones(NPAD, np.float32)
    dinv_pad[:N] = dinv

    xpad = np.zeros((NPAD, D), np.float32)
    xpad[:N] = np.asarray(feat, np.float32)

    # permute into storage-row order
    perm = np.empty(NPAD, np.int64)  # perm[row] = natural node
    perm[row_of] = np.arange(NPAD)
    x_rows = xpad[perm]
    dinv_rows = dinv_pad[perm]

    xg = np.zeros((NPAD, 2 * D), ml_dtypes.bfloat16)
    xg[:, :D] = (dinv_rows[:, None] * x_rows).astype(ml_dtypes.bfloat16)

    in_maps = []
    for c in range(NCORES):
        t = tables[c]
        blk = slice(c * ROWS, (c + 1) * ROWS)
        dinvb = (
            dinv_rows[blk]
            .reshape(WPC, 128)
            .T.reshape(128, WPC, 1)
            .repeat(D, axis=2)
            .reshape(128, WPC * D)
            .astype(np.float32)
        )
        in_maps.append(
            {
                "xg": xg,
                "x0own": x_rows[blk],
                "dinvb": dinvb,
                "idx1": t["idx1"],
                "dstl1": t["dstl1"],
                "idx2": t["idx2"],
                "dstl2": t["dstl2"],
            }
        )
    return row_of, GL, GH, G2, in_maps


_CACHE = {}


def _get_program(feat, src, dst, lambda_max):
    re_norm = float(
        np.float32(2.0) / np.asarray(lambda_max, np.float32).reshape(-1)[0]
    )
    key0 = (id(feat), id(src), id(dst), re_norm)
    if _CACHE.get("inkey") != key0:
        _CACHE["inmaps"] = _make_in_maps(feat, src, dst, re_norm)
        _CACHE["inkey"] = key0
    row_of, GL, GH, G2, in_maps = _CACHE["inmaps"]
    key = (GL, GH, G2, re_norm)
    if key not in _CACHE:
        _CACHE[key] = _build_bass(GL, GH, G2, re_norm)
    return _CACHE[key], in_maps, row_of


def kernel(feat, src, dst, lambda_max):
    from concourse.bass_utils import run_bass_kernel_spmd

    nc, in_maps, row_of = _get_program(feat, src, dst, lambda_max)
    res = run_bass_kernel_spmd(nc, in_maps, core_ids=list(range(NCORES)))
    kernel.last_exec_time_ns = res.exec_time_ns
    out_rows = np.concatenate(
        [res.results[c]["out"] for c in range(NCORES)], axis=0
    )
    return np.ascontiguousarray(out_rows[row_of[:N]])


# revision 30
# speedup vs baseline: 1.8506x; 1.0729x over previous
"""ChebConv (K=3) GNN message passing on 8 Trainium2 NeuronCores.

Strategy (v2):
  - Nodes are permuted into NPAD=61440 storage rows (8 blocks of ROWS=7680,
    60 windows of 128 per block) by a host-side greedy balancer that evens
    out per-window edge counts for both rounds (minimizes tile padding).
  - D^-1/2 normalization is folded into the gathered feature rows
    (x' = dinv*x, host-side for round 1, fused on-device for round 2) and a
    per-dst-node multiply after aggregation; one-hot lhsT tiles are pure 0/1
    indicators (padding slots get an out-of-range dst-local id -> zero col).
  - Round 1 groups edges by DST core (full x0 is resident on every core, so
    no communication): thin dma_gathers (128B payload / 256B stride) fetch
    x'[src] rows; per 6-window PSUM group, one-hot matmuls accumulate the
    segment-sum, then X1 = a1*dinv.h + b1*X0 node-locally.
  - Round 2 groups edges by SRC core (x1[src] is then core-local): each core
    computes partial h2 for ALL 480 global windows from its own X1 block,
    writes bf16 partials to DRAM, and a single ReduceScatter(add) both sums
    the partials and delivers each core its own dst block. The only
    collective is that RS (~8x smaller output than an X1 AllGather).
  - Post phase: X2 = a2*dinv.h2 + b2*X1 - X0, relu, concat, output.
"""

import sys

for _p in ("/opt/trn_rl_repo",):
    if _p not in sys.path:
        sys.path.insert(0, _p)

import numpy as np

# Problem shape (hardcoded per contract).
N, E, D = 60000, 1200000, 64
NCORES = 8
WIN = 128             # dst nodes per window (PSUM partition dim)
WPC = 60              # windows per core
ROWS = WPC * WIN      # 7680 rows per core block
NPAD = NCORES * ROWS  # 61440 padded node rows
NWIN = NCORES * WPC   # 480 global windows
SPLIT = 32768         # int16 gather index range per source half (round 1)
CW = 6                # round-1 windows per chunk (and PSUM group size)
CW2 = 30              # round-2 windows per chunk (5 PSUM groups of 6)
GRP = 6               # windows per PSUM group
PAD_DSTL = 300.0      # out-of-range dst-local id for padding slots


def _dma_gather_thin(
    gp,
    out_ap,
    in_ap,
    idxs_ap,
    num_idxs,
    num_idxs_reg,
    elem_size,
    elem_step,
    queue_num=0,
    single_packet=False,
):
    """dma_gather with payload < 256B (elem_size*dtype need not be a 256B
    multiple); the source row stride (elem_step*dtype) still must be."""
    import concourse.mybir as mybir
    from concourse import ap_utils

    gp._assert_queue_num(queue_num)
    assert idxs_ap.dtype == mybir.dt.int16
    assert in_ap.dtype == out_ap.dtype
    assert in_ap.space.name == "DRAM"
    assert idxs_ap.space.name == "SBUF"
    assert out_ap.space.name == "SBUF"
    assert ap_utils.ap_is_contiguous(out_ap.ap[1:])
    assert ap_utils.ap_is_contiguous(idxs_ap.ap[1:])
    assert in_ap.ap[-1][1] == out_ap.ap[-1][1] == elem_size
    assert out_ap.ap[0][1] * out_ap.ap[1][1] == ((num_idxs + 127) // 128) * 128
    assert in_ap.ap[0][0] == elem_step
    stride_bytes = elem_step * mybir.dt.size(in_ap.dtype)
    assert stride_bytes % 256 == 0
    stride_bytes_256 = stride_bytes // 256
    assert stride_bytes_256 < 256

    _in_ap = gp.lower_ap_dma(in_ap, for_custom_bir_dma=True)
    _idxs_ap = gp.lower_ap(idxs_ap)
    _out_ap = gp.lower_ap(out_ap)
    return gp.add_instruction(
        mybir.InstDMAGatherAnt(
            name=gp.bass.get_next_instruction_name(),
            ins=[
                *_in_ap,
                _idxs_ap,
                gp.lower_val_access(gp.to_reg(num_idxs_reg)),
            ],
            outs=[_out_ap],
            transpose=False,
            num_idxs=num_idxs,
            elem_size=elem_size,
            stride_bytes_256=stride_bytes_256,
            gen_mode=0,
            single_packet=single_packet,
            queue_num=queue_num,
            sbuf_tokens_per_rank=0,
            sbuf_free_dim_per_rank=0,
            sbuf_free_dim_pad_per_rank=0,
            sbuf_byte_offset=0,
        )
    )


def _balance_rows(src, dst):
    """Greedy node->storage-row permutation balancing per-window edge counts.

    Keeps each node in its natural core block; within a block distributes the
    7680 nodes over 60 windows to even out (a) in-edge counts split by src
    half (round-1 low/high gather streams) and (b) in-edge counts split by
    src core (round-2 partial aggregation cells).

    Returns row_of[NPAD]: natural node id -> storage row.
    """
    core_of_nat = np.minimum(np.arange(NPAD) // ROWS, NCORES - 1)
    src_core = np.minimum(src // ROWS, NCORES - 1)

    # provisional half classification by natural id (exact except for core-4
    # srcs whose final row may cross SPLIT; core 4 is balanced first so its
    # final rows are used for every other block)
    row_of = np.arange(NPAD, dtype=np.int64)

    def node_features(c, row_of):
        nodes = np.arange(c * ROWS, (c + 1) * ROWS, dtype=np.int64)
        # in-edges by src core: M[i, j] = #edges (src in core j) -> node i
        sel = np.minimum(dst // ROWS, NCORES - 1) == c
        d_loc = dst[sel] - c * ROWS
        F = np.zeros((ROWS, 10), np.int64)
        np.add.at(F, (d_loc, src_core[sel]), 1)
        low = row_of[src[sel]] < SPLIT
        np.add.at(F, (d_loc[low], 8), 1)
        np.add.at(F, (d_loc[~low], 9), 1)
        return nodes, F

    order_c = [4] + [c for c in range(NCORES) if c != 4]
    for c in order_c:
        nodes, F = node_features(c, row_of)
        tot = F.sum(axis=0).astype(np.float64)
        target = np.maximum(tot / WPC, 1.0)
        # process nodes by total degree, heaviest first
        deg_tot = F[:, :8].sum(axis=1)
        nd_order = np.argsort(-deg_tot, kind="stable")
        W = np.zeros((WPC, 10), np.float64)
        nfill = np.zeros(WPC, np.int64)
        win_of = np.zeros(ROWS, np.int64)
        Fn = F.astype(np.float64)
        for i in nd_order:
            score = ((W + Fn[i]) / target).max(axis=1)
            score[nfill >= WIN] = np.inf
            w = int(np.argmin(score))
            win_of[i] = w
            W[w] += Fn[i]
            nfill[w] += 1
        # rows: window-major, slot order by node id within window
        idx_sorted = np.lexsort((np.arange(ROWS), win_of))
        rows = c * ROWS + np.arange(ROWS)
        row_of[nodes[idx_sorted]] = rows
    return row_of


def _preprocess(src, dst):
    """Build balanced permutation + per-core gather/one-hot tables."""
    import ml_dtypes
    global _bf16np
    _bf16np = ml_dtypes.bfloat16
    src = np.asarray(src, dtype=np.int64)
    dst = np.asarray(dst, dtype=np.int64)

    row_of = _balance_rows(src, dst)
    srcr = row_of[src]
    dstr = row_of[dst]

    gwin = dstr // WIN
    dcore = gwin // WPC
    wloc = gwin % WPC
    scoreid = srcr // ROWS
    low = srcr < SPLIT
    dstl = (dstr % WIN).astype(np.float32)

    # ---- Round 1 (dst-grouped) ----
    nlo = np.zeros((NCORES, WPC), np.int64)
    nhi = np.zeros((NCORES, WPC), np.int64)
    np.add.at(nlo, (dcore[low], wloc[low]), 1)
    np.add.at(nhi, (dcore[~low], wloc[~low]), 1)
    GL = int(np.max((nlo + 127) // 128))
    GH = int(np.max((nhi + 127) // 128))
    T1 = WPC * (GL + GH)
    S1 = T1 * 128
    HI0 = WPC * GL * 128

    key1 = (dcore * WPC + wloc) * 2 + (~low)
    order1 = np.argsort(key1 * (1 << 17) + srcr, kind="stable")
    s1_src = srcr[order1]
    s1_dstl = dstl[order1]
    counts1 = np.zeros(NCORES * WPC * 2, np.int64)
    np.add.at(counts1, key1, 1)
    starts1 = np.concatenate([[0], np.cumsum(counts1)])

    # ---- Round 2 (src-grouped over all 480 global windows) ----
    cnt2 = np.zeros((NCORES, NWIN), np.int64)
    np.add.at(cnt2, (scoreid, gwin), 1)
    G2 = int(np.max((cnt2 + 127) // 128))
    T2 = NWIN * G2
    S2 = T2 * 128

    key2 = scoreid * NWIN + gwin
    order2 = np.argsort(key2 * (1 << 13) + (srcr % ROWS), kind="stable")
    s2_src = (srcr % ROWS)[order2]
    s2_dstl = dstl[order2]
    counts2 = np.zeros(NCORES * NWIN, np.int64)
    np.add.at(counts2, key2, 1)
    starts2 = np.concatenate([[0], np.cumsum(counts2)])

    tables = []
    for c in range(NCORES):
        idx1 = np.zeros(S1, np.int16)
        dstl1 = np.full(S1, PAD_DSTL, np.float32)
        for w in range(WPC):
            kbase = (c * WPC + w) * 2
            a, b = starts1[kbase], starts1[kbase + 1]
            n = b - a
            o = w * GL * 128
            if n:
                idx1[o:o + n] = s1_src[a:b].astype(np.int16)
                dstl1[o:o + n] = s1_dstl[a:b]
            a, b = starts1[kbase + 1], starts1[kbase + 2]
            n = b - a
            o = HI0 + w * GH * 128
            if n:
                idx1[o:o + n] = (s1_src[a:b] - SPLIT).astype(np.int16)
                dstl1[o:o + n] = s1_dstl[a:b]

        idx2 = np.zeros(S2, np.int16)
        dstl2 = np.full(S2, PAD_DSTL, np.float32)
        for g in range(NWIN):
            k = c * NWIN + g
            a, b = starts2[k], starts2[k + 1]
            n = b - a
            o = g * G2 * 128
            if n:
                idx2[o:o + n] = s2_src[a:b].astype(np.int16)
                dstl2[o:o + n] = s2_dstl[a:b]

        tables.append(
            {
                "idx1": np.tile(idx1.reshape(S1 // 16, 16).T, (8, 1)),
                "dstl1": dstl1.reshape(T1, 128).T.copy(),
                "idx2": np.tile(idx2.reshape(S2 // 16, 16).T, (8, 1)),
                "dstl2": dstl2.reshape(T2, 128).T.copy(),
            }
        )
    return row_of, GL, GH, G2, tables


def _build_bass(GL, GH, G2, re_norm):
    import concourse.bass as bass
    import concourse.bacc as bacc
    import concourse.mybir as mybir
    import concourse.tile as tile
    import ml_dtypes
    from contextlib import ExitStack

    f32 = mybir.dt.float32
    i16 = mybir.dt.int16
    bf16 = mybir.dt.bfloat16
    AF = mybir.ActivationFunctionType
    OP = mybir.AluOpType

    G = GL + GH
    T1 = WPC * G
    S1 = T1 * 128
    HI0 = WPC * GL * 128
    T2 = NWIN * G2
    S2 = T2 * 128
    NGRP = WPC // GRP          # per-core window groups (10)
    OHT = max(G, GRP * G2)     # one-hot tiles per group allocation

    a1 = float(-re_norm)       # X1 = a1*dinv.h1 + b1*X0
    b1 = float(re_norm - 1.0)
    a2 = float(-2.0 * re_norm)  # X2 = a2*dinv.h2 + b2*X1 - X0
    b2 = float(2.0 * (re_norm - 1.0))

    nc = bacc.Bacc(
        "TRN2",
        target_bir_lowering=False,
        debug=False,
        enable_asserts=False,
        num_devices=NCORES,
        num_swdge_queues=2,
    )
    xg = nc.dram_tensor("xg", [NPAD, 2 * D], bf16, kind="ExternalInput")
    x0own = nc.dram_tensor("x0own", [ROWS, D], f32, kind="ExternalInput")
    dinvb_d = nc.dram_tensor("dinvb", [128, WPC * D], f32, kind="ExternalInput")
    idx1_d = nc.dram_tensor("idx1", [128, S1 // 16], i16, kind="ExternalInput")
    dstl1_d = nc.dram_tensor("dstl1", [128, T1], f32, kind="ExternalInput")
    idx2_d = nc.dram_tensor("idx2", [128, S2 // 16], i16, kind="ExternalInput")
    dstl2_d = nc.dram_tensor("dstl2", [128, T2], f32, kind="ExternalInput")
    out_d = nc.dram_tensor("out", [ROWS, 3 * D], f32, kind="ExternalOutput")
    iota_d = nc.inline_tensor(
        np.broadcast_to(np.arange(128), (128, 128)).astype(ml_dtypes.bfloat16),
        name="iota",
    )

    with ExitStack() as ctx:
        tc = ctx.enter_context(tile.TileContext(nc))
        dram = ctx.enter_context(tc.tile_pool(name="dram", bufs=1, space="DRAM"))
        x1g = dram.tile([ROWS, 2 * D], bf16)
        hparts = [
            dram.tile([NPAD // 2, D], bf16, name=f"hpart{h}") for h in range(2)
        ]
        hrss = [
            dram.tile([ROWS // 2, D], bf16, name=f"hrs{h}") for h in range(2)
        ]

        cpool = ctx.enter_context(tc.tile_pool(name="const", bufs=1))
        # critical-path loads first (gate the first gather / first one-hots)
        idx1_sb = cpool.tile([128, S1 // 16], i16)
        nc.sync.dma_start(out=idx1_sb[:], in_=idx1_d[:])
        dstl1_sb = cpool.tile([128, T1], f32)
        iota_sb = cpool.tile([128, 128], bf16)
        nc.scalar.dma_start(out=iota_sb[:], in_=iota_d[:])
        nc.scalar.dma_start(out=dstl1_sb[:], in_=dstl1_d[:])
        # the rest on other queues / later
        idx2_sb = cpool.tile([128, S2 // 16], i16)
        nc.sync.dma_start(out=idx2_sb[:], in_=idx2_d[:])
        dstl2_sb = cpool.tile([128, T2], f32)
        nc.scalar.dma_start(out=dstl2_sb[:], in_=dstl2_d[:])
        dinvb_sb = cpool.tile([128, WPC * D], f32)
        nc.scalar.dma_start(out=dinvb_sb[:], in_=dinvb_d[:])
        a2dinvb_sb = cpool.tile([128, WPC * D], f32)
        nc.vector.tensor_scalar(
            out=a2dinvb_sb[:], in0=dinvb_sb[:], scalar1=a2, scalar2=None,
            op0=OP.mult,
        )
        xc_sb = cpool.tile([128, WPC * D], f32)
        x0_sb = cpool.tile([128, WPC * D], f32)
        nc.scalar.dma_start(
            out=x0_sb[:].rearrange("p (w d) -> p w d", d=D),
            in_=x0own[:].rearrange("(w p) d -> p w d", p=128),
        )
        x1_sb = cpool.tile([128, WPC * D], f32)
        h2_sb = cpool.tile([128, WPC * D], bf16)

        ohpool = ctx.enter_context(tc.tile_pool(name="oh", bufs=4))
        pspool = ctx.enter_context(tc.tile_pool(name="ps", bufs=6, space="PSUM"))

        L1lo = CW * GL * 128
        L1hi = CW * GH * 128
        L2 = CW2 * G2 * 128
        reg1lo = nc.gpsimd.alloc_register("n_idx_1lo")
        nc.gpsimd.reg_mov(reg1lo, L1lo)
        if L1hi != L1lo:
            reg1hi = nc.gpsimd.alloc_register("n_idx_1hi")
            nc.gpsimd.reg_mov(reg1hi, L1hi)
        else:
            reg1hi = reg1lo
        reg2 = nc.gpsimd.alloc_register("n_idx_2")
        nc.gpsimd.reg_mov(reg2, L2)

        def onehot(ohg, ti, dstl_sb, sti):
            nc.vector.tensor_scalar(
                out=ohg[:, ti, :],
                in0=iota_sb[:],
                scalar1=dstl_sb[:, sti:sti + 1],
                scalar2=None,
                op0=OP.is_equal,
            )
            return ohg[:, ti, :]

        # ================= Round 1: dst-grouped =================
        r1ctx = ExitStack()
        gpool = r1ctx.enter_context(tc.tile_pool(name="gath1", bufs=2))
        mpool = r1ctx.enter_context(tc.tile_pool(name="misc1", bufs=2))
        opool = r1ctx.enter_context(tc.tile_pool(name="outs1", bufs=2))
        for wlo in range(0, WPC, CW):
            glow = gpool.tile([128, CW * GL, D], bf16, tag="glow")
            ghigh = gpool.tile([128, CW * GH, D], bf16, tag="ghigh")
            s0 = wlo * GL * 128
            _dma_gather_thin(
                nc.gpsimd,
                out_ap=glow[:, :, :],
                in_ap=xg[0:SPLIT, 0:D],
                idxs_ap=idx1_sb[:, s0 // 16:(s0 + L1lo) // 16],
                num_idxs=L1lo,
                num_idxs_reg=reg1lo,
                elem_size=D,
                elem_step=2 * D,
            )
            s0h = HI0 + wlo * GH * 128
            _dma_gather_thin(
                nc.gpsimd,
                out_ap=ghigh[:, :, :],
                in_ap=xg[SPLIT:NPAD, 0:D],
                idxs_ap=idx1_sb[:, s0h // 16:(s0h + L1hi) // 16],
                num_idxs=L1hi,
                num_idxs_reg=reg1hi,
                elem_size=D,
                elem_step=2 * D,
                queue_num=1,
            )
            ps = pspool.tile([128, CW * D], f32)
            for wi in range(CW):
                w = wlo + wi
                ohg = ohpool.tile([128, OHT, 128], bf16, tag="ohg")
                for t in range(G):
                    if t < GL:
                        g_ap = glow[:, wi * GL + t, :]
                        sti = w * GL + t
                    else:
                        g_ap = ghigh[:, wi * GH + (t - GL), :]
                        sti = WPC * GL + w * GH + (t - GL)
                    nc.tensor.matmul(
                        ps[:, wi * D:(wi + 1) * D],
                        lhsT=onehot(ohg, t, dstl1_sb, sti),
                        rhs=g_ap,
                        start=(t == 0),
                        stop=(t == G - 1),
                    )
            # group combine: X1 = a1*dinv.h + b1*X0 ; x1g' = dinv.X1 (bf16)
            grp = slice(wlo * D, (wlo + CW) * D)
            dh = mpool.tile([128, CW * D], f32, tag="dh")
            nc.vector.tensor_tensor(
                out=dh[:], in0=ps[:], in1=dinvb_sb[:, grp], op=OP.mult
            )
            tmp = mpool.tile([128, CW * D], f32, tag="t1")
            nc.scalar.activation(tmp[:], dh[:], AF.Copy, scale=a1)
            if b1 == 1.0:
                nc.vector.tensor_tensor(
                    out=x1_sb[:, grp], in0=tmp[:], in1=x0_sb[:, grp], op=OP.add
                )
            else:
                xb = mpool.tile([128, CW * D], f32, tag="t2")
                nc.scalar.activation(xb[:], x0_sb[:, grp], AF.Copy, scale=b1)
                nc.vector.tensor_tensor(
                    out=x1_sb[:, grp], in0=tmp[:], in1=xb[:], op=OP.add
                )
            x1bf = opool.tile([128, CW * D], bf16, tag="x1bf")
            nc.vector.tensor_tensor(
                out=x1bf[:], in0=x1_sb[:, grp], in1=dinvb_sb[:, grp], op=OP.mult
            )
            nc.sync.dma_start(
                out=x1g[wlo * 128:(wlo + CW) * 128, 0:D].rearrange(
                    "(w p) d -> p w d", p=128
                ),
                in_=x1bf[:].rearrange("p (w d) -> p w d", d=D),
            )
            # pre-compute xc = b2*X1 - X0 for the post phase
            xb2 = mpool.tile([128, CW * D], f32, tag="xb2")
            nc.scalar.activation(xb2[:], x1_sb[:, grp], AF.Copy, scale=b2)
            nc.vector.tensor_tensor(
                out=xc_sb[:, grp], in0=xb2[:], in1=x0_sb[:, grp], op=OP.subtract
            )

        r1ctx.close()

        # ================= Round 2: src-grouped partials =================
        # Global windows processed in half-interleaved order: pass h covers
        # windows [c*WPC + h*HWPC, +HWPC) of every dst core c, so each pass
        # fills one contiguous replica-ordered partial buffer and its
        # ReduceScatter can overlap the other pass / the post phase.
        r2ctx = ExitStack()
        gpool2 = r2ctx.enter_context(tc.tile_pool(name="gath2", bufs=2))
        opool2 = r2ctx.enter_context(tc.tile_pool(name="outs2", bufs=2))
        HWPC = WPC // 2
        ci = 0
        for h in range(2):
            for c8 in range(NCORES):
                glo = c8 * WPC + h * HWPC
                g2 = gpool2.tile([128, CW2 * G2, D], bf16, tag="g2")
                s0 = glo * G2 * 128
                _dma_gather_thin(
                    nc.gpsimd,
                    out_ap=g2[:, :, :],
                    in_ap=x1g[0:ROWS, 0:D],
                    idxs_ap=idx2_sb[:, s0 // 16:(s0 + L2) // 16],
                    num_idxs=L2,
                    num_idxs_reg=reg2,
                    elem_size=D,
                    elem_step=2 * D,
                    queue_num=ci % 2,
                )
                ci += 1
                hst = opool2.tile([128, CW2 * D], bf16, tag="hst")
                for gi in range(0, CW2, GRP):
                    ps = pspool.tile([128, GRP * D], f32)
                    ohg = ohpool.tile([128, OHT, 128], bf16, tag="ohg")
                    for wi in range(GRP):
                        gw = glo + gi + wi
                        for t in range(G2):
                            nc.tensor.matmul(
                                ps[:, wi * D:(wi + 1) * D],
                                lhsT=onehot(
                                    ohg, wi * G2 + t, dstl2_sb, gw * G2 + t
                                ),
                                rhs=g2[:, (gi + wi) * G2 + t, :],
                                start=(t == 0),
                                stop=(t == G2 - 1),
                            )
                    nc.scalar.activation(
                        hst[:, gi * D:(gi + GRP) * D], ps[:], AF.Copy
                    )
                hp = hparts[h]
                nc.sync.dma_start(
                    out=hp[c8 * CW2 * 128:(c8 + 1) * CW2 * 128, :].rearrange(
                        "(w p) d -> p w d", p=128
                    ),
                    in_=hst[:].rearrange("p (w d) -> p w d", d=D),
                )
            # half-ReduceScatter as soon as this pass's partials are written
            nc.gpsimd.collective_compute(
                "ReduceScatter",
                mybir.AluOpType.add,
                replica_groups=[list(range(NCORES))],
                ins=[hparts[h][:]],
                outs=[hrss[h][:]],
            )

        r2ctx.close()

        # ================= Post: X2 + relu + output =================
        mpool = ctx.enter_context(tc.tile_pool(name="misc3", bufs=3))
        opool = ctx.enter_context(tc.tile_pool(name="outs3", bufs=3))
        for h in range(2):
            nc.sync.dma_start(
                out=h2_sb[:, h * HWPC * D:(h + 1) * HWPC * D].rearrange(
                    "p (w d) -> p w d", d=D
                ),
                in_=hrss[h][:].rearrange("(w p) d -> p w d", p=128),
            )
        for gi in range(NGRP):
            grp = slice(gi * GRP * D, (gi + 1) * GRP * D)
            t1 = mpool.tile([128, GRP * D], f32, tag="t1b")
            nc.vector.tensor_tensor(
                out=t1[:], in0=h2_sb[:, grp], in1=a2dinvb_sb[:, grp], op=OP.mult
            )
            x2 = mpool.tile([128, GRP * D], f32, tag="x2")
            nc.vector.tensor_tensor(
                out=x2[:], in0=t1[:], in1=xc_sb[:, grp], op=OP.add
            )
            outg = opool.tile([128, GRP * 3 * D], f32, tag="outg")
            for wi in range(GRP):
                ob = wi * 3 * D
                ws = slice((gi * GRP + wi) * D, (gi * GRP + wi + 1) * D)
                nc.scalar.activation(outg[:, ob:ob + D], x0_sb[:, ws], AF.Relu)
                nc.scalar.activation(
                    outg[:, ob + D:ob + 2 * D], x1_sb[:, ws], AF.Relu
                )
                nc.scalar.activation(
                    outg[:, ob + 2 * D:ob + 3 * D],
                    x2[:, wi * D:(wi + 1) * D],
                    AF.Relu,
                )
            nc.sync.dma_start(
                out=out_d[gi * GRP * 128:(gi + 1) * GRP * 128, :].rearrange(
                    "(w p) d -> p w d", p=128
                ),
                in_=outg[:].rearrange("p (w d) -> p w d", d=3 * D),
            )

    nc.finalize()
    return nc


def _make_in_maps(feat, src, dst, re_norm):
    import ml_dtypes

    row_of, GL, GH, G2, tables = _preprocess(src, dst)

    deg = np.bincount(np.asarray(dst, np.int64), minlength=N).astype(np.float32)
    dinv = np.maximum(deg, 1.0) ** -0.5
    dinv_pad = np.ones(NPAD, np.float32)
    dinv_pad[:N] = dinv

    xpad = np.zeros((NPAD, D), np.float32)
    xpad[:N] = np.asarray(feat, np.float32)

    # permute into storage-row order
    perm = np.empty(NPAD, np.int64)  # perm[row] = natural node
    perm[row_of] = np.arange(NPAD)
    x_rows = xpad[perm]
    dinv_rows = dinv_pad[perm]

    xg = np.zeros((NPAD, 2 * D), ml_dtypes.bfloat16)
    xg[:, :D] = (dinv_rows[:, None] * x_rows).astype(ml_dtypes.bfloat16)

    in_maps = []
    for c in range(NCORES):
        t = tables[c]
        blk = slice(c * ROWS, (c + 1) * ROWS)
        dinvb = (
            dinv_rows[blk]
            .reshape(WPC, 128)
            .T.reshape(128, WPC, 1)
            .repeat(D, axis=2)
            .reshape(128, WPC * D)
            .astype(np.float32)
        )
        in_maps.append(
            {
                "xg": xg,
                "x0own": x_rows[blk],
                "dinvb": dinvb,
                "idx1": t["idx1"],
                "dstl1": t["dstl1"],
                "idx2": t["idx2"],
                "dstl2": t["dstl2"],
            }
        )
    return row_of, GL, GH, G2, in_maps


_CACHE = {}


def _get_program(feat, src, dst, lambda_max):
    re_norm = float(
        np.float32(2.0) / np.asarray(lambda_max, np.float32).reshape(-1)[0]
    )
    key0 = (id(feat), id(src), id(dst), re_norm)
    if _CACHE.get("inkey") != key0:
        _CACHE["inmaps"] = _make_in_maps(feat, src, dst, re_norm)
        _CACHE["inkey"] = key0
    row_of, GL, GH, G2, in_maps = _CACHE["inmaps"]
    key = (GL, GH, G2, re_norm)
    if key not in _CACHE:
        _CACHE[key] = _build_bass(GL, GH, G2, re_norm)
    return _CACHE[key], in_maps, row_of


def kernel(feat, src, dst, lambda_max):
    from concourse.bass_utils import run_bass_kernel_spmd

    nc, in_maps, row_of = _get_program(feat, src, dst, lambda_max)
    res = run_bass_kernel_spmd(nc, in_maps, core_ids=list(range(NCORES)))
    kernel.last_exec_time_ns = res.exec_time_ns
    out_rows = np.concatenate(
        [res.results[c]["out"] for c in range(NCORES)], axis=0
    )
    return np.ascontiguousarray(out_rows[row_of[:N]])


# revision 31
# speedup vs baseline: 1.8626x; 1.0065x over previous
"""ChebConv (K=3) GNN message passing on 8 Trainium2 NeuronCores.

Strategy (v2):
  - Nodes are permuted into NPAD=61440 storage rows (8 blocks of ROWS=7680,
    60 windows of 128 per block) by a host-side greedy balancer that evens
    out per-window edge counts for both rounds (minimizes tile padding).
  - D^-1/2 normalization is folded into the gathered feature rows
    (x' = dinv*x, host-side for round 1, fused on-device for round 2) and a
    per-dst-node multiply after aggregation; one-hot lhsT tiles are pure 0/1
    indicators (padding slots get an out-of-range dst-local id -> zero col).
  - Round 1 groups edges by DST core (full x0 is resident on every core, so
    no communication): thin dma_gathers (128B payload / 256B stride) fetch
    x'[src] rows; per 6-window PSUM group, one-hot matmuls accumulate the
    segment-sum, then X1 = a1*dinv.h + b1*X0 node-locally.
  - Round 2 groups edges by SRC core (x1[src] is then core-local): each core
    computes partial h2 for ALL 480 global windows from its own X1 block,
    writes bf16 partials to DRAM, and a single ReduceScatter(add) both sums
    the partials and delivers each core its own dst block. The only
    collective is that RS (~8x smaller output than an X1 AllGather).
  - Post phase: X2 = a2*dinv.h2 + b2*X1 - X0, relu, concat, output.
"""

import sys

for _p in ("/opt/trn_rl_repo",):
    if _p not in sys.path:
        sys.path.insert(0, _p)

import numpy as np

# Problem shape (hardcoded per contract).
N, E, D = 60000, 1200000, 64
NCORES = 8
WIN = 128             # dst nodes per window (PSUM partition dim)
WPC = 60              # windows per core
ROWS = WPC * WIN      # 7680 rows per core block
NPAD = NCORES * ROWS  # 61440 padded node rows
NWIN = NCORES * WPC   # 480 global windows
SPLIT = 32768         # int16 gather index range per source half (round 1)
CW = 6                # round-1 windows per chunk (and PSUM group size)
CW2 = 30              # round-2 windows per chunk (5 PSUM groups of 6)
GRP = 6               # windows per PSUM group
PAD_DSTL = 300.0      # out-of-range dst-local id for padding slots


def _dma_gather_thin(
    gp,
    out_ap,
    in_ap,
    idxs_ap,
    num_idxs,
    num_idxs_reg,
    elem_size,
    elem_step,
    queue_num=0,
    single_packet=False,
):
    """dma_gather with payload < 256B (elem_size*dtype need not be a 256B
    multiple); the source row stride (elem_step*dtype) still must be."""
    import concourse.mybir as mybir
    from concourse import ap_utils

    gp._assert_queue_num(queue_num)
    assert idxs_ap.dtype == mybir.dt.int16
    assert in_ap.dtype == out_ap.dtype
    assert in_ap.space.name == "DRAM"
    assert idxs_ap.space.name == "SBUF"
    assert out_ap.space.name == "SBUF"
    assert ap_utils.ap_is_contiguous(out_ap.ap[1:])
    assert ap_utils.ap_is_contiguous(idxs_ap.ap[1:])
    assert in_ap.ap[-1][1] == out_ap.ap[-1][1] == elem_size
    assert out_ap.ap[0][1] * out_ap.ap[1][1] == ((num_idxs + 127) // 128) * 128
    assert in_ap.ap[0][0] == elem_step
    stride_bytes = elem_step * mybir.dt.size(in_ap.dtype)
    assert stride_bytes % 256 == 0
    stride_bytes_256 = stride_bytes // 256
    assert stride_bytes_256 < 256

    _in_ap = gp.lower_ap_dma(in_ap, for_custom_bir_dma=True)
    _idxs_ap = gp.lower_ap(idxs_ap)
    _out_ap = gp.lower_ap(out_ap)
    return gp.add_instruction(
        mybir.InstDMAGatherAnt(
            name=gp.bass.get_next_instruction_name(),
            ins=[
                *_in_ap,
                _idxs_ap,
                gp.lower_val_access(gp.to_reg(num_idxs_reg)),
            ],
            outs=[_out_ap],
            transpose=False,
            num_idxs=num_idxs,
            elem_size=elem_size,
            stride_bytes_256=stride_bytes_256,
            gen_mode=0,
            single_packet=single_packet,
            queue_num=queue_num,
            sbuf_tokens_per_rank=0,
            sbuf_free_dim_per_rank=0,
            sbuf_free_dim_pad_per_rank=0,
            sbuf_byte_offset=0,
        )
    )


def _balance_rows(src, dst):
    """Greedy node->storage-row permutation balancing per-window edge counts.

    Keeps each node in its natural core block; within a block distributes the
    7680 nodes over 60 windows to even out (a) in-edge counts split by src
    half (round-1 low/high gather streams) and (b) in-edge counts split by
    src core (round-2 partial aggregation cells).

    Returns row_of[NPAD]: natural node id -> storage row.
    """
    core_of_nat = np.minimum(np.arange(NPAD) // ROWS, NCORES - 1)
    src_core = np.minimum(src // ROWS, NCORES - 1)

    # provisional half classification by natural id (exact except for core-4
    # srcs whose final row may cross SPLIT; core 4 is balanced first so its
    # final rows are used for every other block)
    row_of = np.arange(NPAD, dtype=np.int64)

    def node_features(c, row_of):
        nodes = np.arange(c * ROWS, (c + 1) * ROWS, dtype=np.int64)
        # in-edges by src core: M[i, j] = #edges (src in core j) -> node i
        sel = np.minimum(dst // ROWS, NCORES - 1) == c
        d_loc = dst[sel] - c * ROWS
        F = np.zeros((ROWS, 10), np.int64)
        np.add.at(F, (d_loc, src_core[sel]), 1)
        low = row_of[src[sel]] < SPLIT
        np.add.at(F, (d_loc[low], 8), 1)
        np.add.at(F, (d_loc[~low], 9), 1)
        return nodes, F

    order_c = [4] + [c for c in range(NCORES) if c != 4]
    for c in order_c:
        nodes, F = node_features(c, row_of)
        tot = F.sum(axis=0).astype(np.float64)
        target = np.maximum(tot / WPC, 1.0)
        # process nodes by total degree, heaviest first
        deg_tot = F[:, :8].sum(axis=1)
        nd_order = np.argsort(-deg_tot, kind="stable")
        W = np.zeros((WPC, 10), np.float64)
        nfill = np.zeros(WPC, np.int64)
        win_of = np.zeros(ROWS, np.int64)
        Fn = F.astype(np.float64)
        for i in nd_order:
            score = ((W + Fn[i]) / target).max(axis=1)
            score[nfill >= WIN] = np.inf
            w = int(np.argmin(score))
            win_of[i] = w
            W[w] += Fn[i]
            nfill[w] += 1
        # rows: window-major, slot order by node id within window
        idx_sorted = np.lexsort((np.arange(ROWS), win_of))
        rows = c * ROWS + np.arange(ROWS)
        row_of[nodes[idx_sorted]] = rows
    return row_of


def _preprocess(src, dst):
    """Build balanced permutation + per-core gather/one-hot tables."""
    import ml_dtypes
    global _bf16np
    _bf16np = ml_dtypes.bfloat16
    src = np.asarray(src, dtype=np.int64)
    dst = np.asarray(dst, dtype=np.int64)

    row_of = _balance_rows(src, dst)
    srcr = row_of[src]
    dstr = row_of[dst]

    gwin = dstr // WIN
    dcore = gwin // WPC
    wloc = gwin % WPC
    scoreid = srcr // ROWS
    low = srcr < SPLIT
    dstl = (dstr % WIN).astype(np.float32)

    # ---- Round 1 (dst-grouped) ----
    nlo = np.zeros((NCORES, WPC), np.int64)
    nhi = np.zeros((NCORES, WPC), np.int64)
    np.add.at(nlo, (dcore[low], wloc[low]), 1)
    np.add.at(nhi, (dcore[~low], wloc[~low]), 1)
    GL = int(np.max((nlo + 127) // 128))
    GH = int(np.max((nhi + 127) // 128))
    T1 = WPC * (GL + GH)
    S1 = T1 * 128
    HI0 = WPC * GL * 128

    key1 = (dcore * WPC + wloc) * 2 + (~low)
    order1 = np.argsort(key1 * (1 << 17) + srcr, kind="stable")
    s1_src = srcr[order1]
    s1_dstl = dstl[order1]
    counts1 = np.zeros(NCORES * WPC * 2, np.int64)
    np.add.at(counts1, key1, 1)
    starts1 = np.concatenate([[0], np.cumsum(counts1)])

    # ---- Round 2 (src-grouped over all 480 global windows) ----
    cnt2 = np.zeros((NCORES, NWIN), np.int64)
    np.add.at(cnt2, (scoreid, gwin), 1)
    G2 = int(np.max((cnt2 + 127) // 128))
    T2 = NWIN * G2
    S2 = T2 * 128

    key2 = scoreid * NWIN + gwin
    order2 = np.argsort(key2 * (1 << 13) + (srcr % ROWS), kind="stable")
    s2_src = (srcr % ROWS)[order2]
    s2_dstl = dstl[order2]
    counts2 = np.zeros(NCORES * NWIN, np.int64)
    np.add.at(counts2, key2, 1)
    starts2 = np.concatenate([[0], np.cumsum(counts2)])

    tables = []
    for c in range(NCORES):
        idx1 = np.zeros(S1, np.int16)
        dstl1 = np.full(S1, PAD_DSTL, np.float32)
        for w in range(WPC):
            kbase = (c * WPC + w) * 2
            a, b = starts1[kbase], starts1[kbase + 1]
            n = b - a
            o = w * GL * 128
            if n:
                idx1[o:o + n] = s1_src[a:b].astype(np.int16)
                dstl1[o:o + n] = s1_dstl[a:b]
            a, b = starts1[kbase + 1], starts1[kbase + 2]
            n = b - a
            o = HI0 + w * GH * 128
            if n:
                idx1[o:o + n] = (s1_src[a:b] - SPLIT).astype(np.int16)
                dstl1[o:o + n] = s1_dstl[a:b]

        idx2 = np.zeros(S2, np.int16)
        dstl2 = np.full(S2, PAD_DSTL, np.float32)
        for g in range(NWIN):
            k = c * NWIN + g
            a, b = starts2[k], starts2[k + 1]
            n = b - a
            o = g * G2 * 128
            if n:
                idx2[o:o + n] = s2_src[a:b].astype(np.int16)
                dstl2[o:o + n] = s2_dstl[a:b]

        tables.append(
            {
                "idx1": np.tile(idx1.reshape(S1 // 16, 16).T, (8, 1)),
                "dstl1": dstl1.reshape(T1, 128).T.copy(),
                "idx2": np.tile(idx2.reshape(S2 // 16, 16).T, (8, 1)),
                "dstl2": dstl2.reshape(T2, 128).T.copy(),
            }
        )
    return row_of, GL, GH, G2, tables


def _build_bass(GL, GH, G2, re_norm):
    import concourse.bass as bass
    import concourse.bacc as bacc
    import concourse.mybir as mybir
    import concourse.tile as tile
    import ml_dtypes
    from contextlib import ExitStack

    f32 = mybir.dt.float32
    i16 = mybir.dt.int16
    bf16 = mybir.dt.bfloat16
    AF = mybir.ActivationFunctionType
    OP = mybir.AluOpType

    G = GL + GH
    T1 = WPC * G
    S1 = T1 * 128
    HI0 = WPC * GL * 128
    T2 = NWIN * G2
    S2 = T2 * 128
    NGRP = WPC // GRP          # per-core window groups (10)
    OHT = max(G, GRP * G2)     # one-hot tiles per group allocation

    a1 = float(-re_norm)       # X1 = a1*dinv.h1 + b1*X0
    b1 = float(re_norm - 1.0)
    a2 = float(-2.0 * re_norm)  # X2 = a2*dinv.h2 + b2*X1 - X0
    b2 = float(2.0 * (re_norm - 1.0))

    nc = bacc.Bacc(
        "TRN2",
        target_bir_lowering=False,
        debug=False,
        enable_asserts=False,
        num_devices=NCORES,
        num_swdge_queues=2,
    )
    xg = nc.dram_tensor("xg", [NPAD, 2 * D], bf16, kind="ExternalInput")
    x0own = nc.dram_tensor("x0own", [ROWS, D], f32, kind="ExternalInput")
    dinvb_d = nc.dram_tensor("dinvb", [128, WPC * D], f32, kind="ExternalInput")
    idx1_d = nc.dram_tensor("idx1", [128, S1 // 16], i16, kind="ExternalInput")
    dstl1_d = nc.dram_tensor("dstl1", [128, T1], f32, kind="ExternalInput")
    idx2_d = nc.dram_tensor("idx2", [128, S2 // 16], i16, kind="ExternalInput")
    dstl2_d = nc.dram_tensor("dstl2", [128, T2], f32, kind="ExternalInput")
    out_d = nc.dram_tensor("out", [ROWS, 3 * D], f32, kind="ExternalOutput")
    iota_d = nc.inline_tensor(
        np.broadcast_to(np.arange(128), (128, 128)).astype(ml_dtypes.bfloat16),
        name="iota",
    )

    with ExitStack() as ctx:
        tc = ctx.enter_context(tile.TileContext(nc))
        dram = ctx.enter_context(tc.tile_pool(name="dram", bufs=1, space="DRAM"))
        x1g = dram.tile([ROWS, 2 * D], bf16)
        hparts = [
            dram.tile([NPAD // 2, D], bf16, name=f"hpart{h}") for h in range(2)
        ]
        hrss = [
            dram.tile([ROWS // 2, D], bf16, name=f"hrs{h}") for h in range(2)
        ]

        cpool = ctx.enter_context(tc.tile_pool(name="const", bufs=1))
        # critical-path loads first (gate the first gather / first one-hots)
        idx1_sb = cpool.tile([128, S1 // 16], i16)
        _c0 = (CW * GL * 128) // 16
        nc.sync.dma_start(out=idx1_sb[:, 0:_c0], in_=idx1_d[:, 0:_c0])
        _h0 = (WPC * GL * 128) // 16
        _h1 = _h0 + (CW * GH * 128) // 16
        nc.sync.dma_start(out=idx1_sb[:, _h0:_h1], in_=idx1_d[:, _h0:_h1])
        nc.sync.dma_start(out=idx1_sb[:, _c0:_h0], in_=idx1_d[:, _c0:_h0])
        nc.sync.dma_start(out=idx1_sb[:, _h1:], in_=idx1_d[:, _h1:])
        dstl1_sb = cpool.tile([128, T1], f32)
        iota_sb = cpool.tile([128, 128], bf16)
        nc.scalar.dma_start(out=iota_sb[:], in_=iota_d[:])
        nc.scalar.dma_start(out=dstl1_sb[:], in_=dstl1_d[:])
        # the rest on other queues / later
        idx2_sb = cpool.tile([128, S2 // 16], i16)
        nc.sync.dma_start(out=idx2_sb[:], in_=idx2_d[:])
        dstl2_sb = cpool.tile([128, T2], f32)
        nc.scalar.dma_start(out=dstl2_sb[:], in_=dstl2_d[:])
        dinvb_sb = cpool.tile([128, WPC * D], f32)
        nc.scalar.dma_start(out=dinvb_sb[:], in_=dinvb_d[:])
        a2dinvb_sb = cpool.tile([128, WPC * D], f32)
        nc.vector.tensor_scalar(
            out=a2dinvb_sb[:], in0=dinvb_sb[:], scalar1=a2, scalar2=None,
            op0=OP.mult,
        )
        xc_sb = cpool.tile([128, WPC * D], f32)
        x0_sb = cpool.tile([128, WPC * D], f32)
        nc.scalar.dma_start(
            out=x0_sb[:].rearrange("p (w d) -> p w d", d=D),
            in_=x0own[:].rearrange("(w p) d -> p w d", p=128),
        )
        x1_sb = cpool.tile([128, WPC * D], f32)
        h2_sb = cpool.tile([128, WPC * D], bf16)

        ohpool = ctx.enter_context(tc.tile_pool(name="oh", bufs=4))
        pspool = ctx.enter_context(tc.tile_pool(name="ps", bufs=6, space="PSUM"))

        L1lo = CW * GL * 128
        L1hi = CW * GH * 128
        L2 = CW2 * G2 * 128
        reg1lo = nc.gpsimd.alloc_register("n_idx_1lo")
        nc.gpsimd.reg_mov(reg1lo, L1lo)
        if L1hi != L1lo:
            reg1hi = nc.gpsimd.alloc_register("n_idx_1hi")
            nc.gpsimd.reg_mov(reg1hi, L1hi)
        else:
            reg1hi = reg1lo
        reg2 = nc.gpsimd.alloc_register("n_idx_2")
        nc.gpsimd.reg_mov(reg2, L2)

        def onehot(ohg, ti, dstl_sb, sti):
            nc.vector.tensor_scalar(
                out=ohg[:, ti, :],
                in0=iota_sb[:],
                scalar1=dstl_sb[:, sti:sti + 1],
                scalar2=None,
                op0=OP.is_equal,
            )
            return ohg[:, ti, :]

        # ================= Round 1: dst-grouped =================
        r1ctx = ExitStack()
        gpool = r1ctx.enter_context(tc.tile_pool(name="gath1", bufs=2))
        mpool = r1ctx.enter_context(tc.tile_pool(name="misc1", bufs=2))
        opool = r1ctx.enter_context(tc.tile_pool(name="outs1", bufs=2))
        for wlo in range(0, WPC, CW):
            glow = gpool.tile([128, CW * GL, D], bf16, tag="glow")
            ghigh = gpool.tile([128, CW * GH, D], bf16, tag="ghigh")
            s0 = wlo * GL * 128
            _dma_gather_thin(
                nc.gpsimd,
                out_ap=glow[:, :, :],
                in_ap=xg[0:SPLIT, 0:D],
                idxs_ap=idx1_sb[:, s0 // 16:(s0 + L1lo) // 16],
                num_idxs=L1lo,
                num_idxs_reg=reg1lo,
                elem_size=D,
                elem_step=2 * D,
            )
            s0h = HI0 + wlo * GH * 128
            _dma_gather_thin(
                nc.gpsimd,
                out_ap=ghigh[:, :, :],
                in_ap=xg[SPLIT:NPAD, 0:D],
                idxs_ap=idx1_sb[:, s0h // 16:(s0h + L1hi) // 16],
                num_idxs=L1hi,
                num_idxs_reg=reg1hi,
                elem_size=D,
                elem_step=2 * D,
                queue_num=1,
            )
            ps = pspool.tile([128, CW * D], f32)
            for wi in range(CW):
                w = wlo + wi
                ohg = ohpool.tile([128, OHT, 128], bf16, tag="ohg")
                for t in range(G):
                    if t < GL:
                        g_ap = glow[:, wi * GL + t, :]
                        sti = w * GL + t
                    else:
                        g_ap = ghigh[:, wi * GH + (t - GL), :]
                        sti = WPC * GL + w * GH + (t - GL)
                    nc.tensor.matmul(
                        ps[:, wi * D:(wi + 1) * D],
                        lhsT=onehot(ohg, t, dstl1_sb, sti),
                        rhs=g_ap,
                        start=(t == 0),
                        stop=(t == G - 1),
                    )
            # group combine: X1 = a1*dinv.h + b1*X0 ; x1g' = dinv.X1 (bf16)
            grp = slice(wlo * D, (wlo + CW) * D)
            dh = mpool.tile([128, CW * D], f32, tag="dh")
            nc.vector.tensor_tensor(
                out=dh[:], in0=ps[:], in1=dinvb_sb[:, grp], op=OP.mult
            )
            tmp = mpool.tile([128, CW * D], f32, tag="t1")
            nc.scalar.activation(tmp[:], dh[:], AF.Copy, scale=a1)
            if b1 == 1.0:
                nc.vector.tensor_tensor(
                    out=x1_sb[:, grp], in0=tmp[:], in1=x0_sb[:, grp], op=OP.add
                )
            else:
                xb = mpool.tile([128, CW * D], f32, tag="t2")
                nc.scalar.activation(xb[:], x0_sb[:, grp], AF.Copy, scale=b1)
                nc.vector.tensor_tensor(
                    out=x1_sb[:, grp], in0=tmp[:], in1=xb[:], op=OP.add
                )
            x1bf = opool.tile([128, CW * D], bf16, tag="x1bf")
            nc.vector.tensor_tensor(
                out=x1bf[:], in0=x1_sb[:, grp], in1=dinvb_sb[:, grp], op=OP.mult
            )
            nc.sync.dma_start(
                out=x1g[wlo * 128:(wlo + CW) * 128, 0:D].rearrange(
                    "(w p) d -> p w d", p=128
                ),
                in_=x1bf[:].rearrange("p (w d) -> p w d", d=D),
            )
            # pre-compute xc = b2*X1 - X0 for the post phase
            xb2 = mpool.tile([128, CW * D], f32, tag="xb2")
            nc.scalar.activation(xb2[:], x1_sb[:, grp], AF.Copy, scale=b2)
            nc.vector.tensor_tensor(
                out=xc_sb[:, grp], in0=xb2[:], in1=x0_sb[:, grp], op=OP.subtract
            )

        r1ctx.close()

        # ================= Round 2: src-grouped partials =================
        # Global windows processed in half-interleaved order: pass h covers
        # windows [c*WPC + h*HWPC, +HWPC) of every dst core c, so each pass
        # fills one contiguous replica-ordered partial buffer and its
        # ReduceScatter can overlap the other pass / the post phase.
        r2ctx = ExitStack()
        gpool2 = r2ctx.enter_context(tc.tile_pool(name="gath2", bufs=3))
        opool2 = r2ctx.enter_context(tc.tile_pool(name="outs2", bufs=2))
        HWPC = WPC // 2
        ci = 0
        for h in range(2):
            for c8 in range(NCORES):
                glo = c8 * WPC + h * HWPC
                g2 = gpool2.tile([128, CW2 * G2, D], bf16, tag="g2")
                s0 = glo * G2 * 128
                _dma_gather_thin(
                    nc.gpsimd,
                    out_ap=g2[:, :, :],
                    in_ap=x1g[0:ROWS, 0:D],
                    idxs_ap=idx2_sb[:, s0 // 16:(s0 + L2) // 16],
                    num_idxs=L2,
                    num_idxs_reg=reg2,
                    elem_size=D,
                    elem_step=2 * D,
                    queue_num=ci % 2,
                )
                ci += 1
                hst = opool2.tile([128, CW2 * D], bf16, tag="hst")
                for gi in range(0, CW2, GRP):
                    ps = pspool.tile([128, GRP * D], f32)
                    ohg = ohpool.tile([128, OHT, 128], bf16, tag="ohg")
                    for wi in range(GRP):
                        gw = glo + gi + wi
                        for t in range(G2):
                            nc.tensor.matmul(
                                ps[:, wi * D:(wi + 1) * D],
                                lhsT=onehot(
                                    ohg, wi * G2 + t, dstl2_sb, gw * G2 + t
                                ),
                                rhs=g2[:, (gi + wi) * G2 + t, :],
                                start=(t == 0),
                                stop=(t == G2 - 1),
                            )
                    nc.scalar.activation(
                        hst[:, gi * D:(gi + GRP) * D], ps[:], AF.Copy
                    )
                hp = hparts[h]
                nc.sync.dma_start(
                    out=hp[c8 * CW2 * 128:(c8 + 1) * CW2 * 128, :].rearrange(
                        "(w p) d -> p w d", p=128
                    ),
                    in_=hst[:].rearrange("p (w d) -> p w d", d=D),
                )
            # half-ReduceScatter as soon as this pass's partials are written
            nc.gpsimd.collective_compute(
                "ReduceScatter",
                mybir.AluOpType.add,
                replica_groups=[list(range(NCORES))],
                ins=[hparts[h][:]],
                outs=[hrss[h][:]],
            )

        r2ctx.close()

        # ================= Post: X2 + relu + output =================
        mpool = ctx.enter_context(tc.tile_pool(name="misc3", bufs=3))
        opool = ctx.enter_context(tc.tile_pool(name="outs3", bufs=3))
        for h in range(2):
            nc.sync.dma_start(
                out=h2_sb[:, h * HWPC * D:(h + 1) * HWPC * D].rearrange(
                    "p (w d) -> p w d", d=D
                ),
                in_=hrss[h][:].rearrange("(w p) d -> p w d", p=128),
            )
        for gi in range(NGRP):
            grp = slice(gi * GRP * D, (gi + 1) * GRP * D)
            t1 = mpool.tile([128, GRP * D], f32, tag="t1b")
            nc.vector.tensor_tensor(
                out=t1[:], in0=h2_sb[:, grp], in1=a2dinvb_sb[:, grp], op=OP.mult
            )
            x2 = mpool.tile([128, GRP * D], f32, tag="x2")
            nc.vector.tensor_tensor(
                out=x2[:], in0=t1[:], in1=xc_sb[:, grp], op=OP.add
            )
            outg = opool.tile([128, GRP * 3 * D], f32, tag="outg")
            for wi in range(GRP):
                ob = wi * 3 * D
                ws = slice((gi * GRP + wi) * D, (gi * GRP + wi + 1) * D)
                nc.scalar.activation(outg[:, ob:ob + D], x0_sb[:, ws], AF.Relu)
                nc.scalar.activation(
                    outg[:, ob + D:ob + 2 * D], x1_sb[:, ws], AF.Relu
                )
                nc.scalar.activation(
                    outg[:, ob + 2 * D:ob + 3 * D],
                    x2[:, wi * D:(wi + 1) * D],
                    AF.Relu,
                )
            nc.sync.dma_start(
                out=out_d[gi * GRP * 128:(gi + 1) * GRP * 128, :].rearrange(
                    "(w p) d -> p w d", p=128
                ),
                in_=outg[:].rearrange("p (w d) -> p w d", d=3 * D),
            )

    nc.finalize()
    return nc


def _make_in_maps(feat, src, dst, re_norm):
    import ml_dtypes

    row_of, GL, GH, G2, tables = _preprocess(src, dst)

    deg = np.bincount(np.asarray(dst, np.int64), minlength=N).astype(np.float32)
    dinv = np.maximum(deg, 1.0) ** -0.5
    dinv_pad = np.ones(NPAD, np.float32)
    dinv_pad[:N] = dinv

    xpad = np.zeros((NPAD, D), np.float32)
    xpad[:N] = np.asarray(feat, np.float32)

    # permute into storage-row order
    perm = np.empty(NPAD, np.int64)  # perm[row] = natural node
    perm[row_of] = np.arange(NPAD)
    x_rows = xpad[perm]
    dinv_rows = dinv_pad[perm]

    xg = np.zeros((NPAD, 2 * D), ml_dtypes.bfloat16)
    xg[:, :D] = (dinv_rows[:, None] * x_rows).astype(ml_dtypes.bfloat16)

    in_maps = []
    for c in range(NCORES):
        t = tables[c]
        blk = slice(c * ROWS, (c + 1) * ROWS)
        dinvb = (
            dinv_rows[blk]
            .reshape(WPC, 128)
            .T.reshape(128, WPC, 1)
            .repeat(D, axis=2)
            .reshape(128, WPC * D)
            .astype(np.float32)
        )
        in_maps.append(
            {
                "xg": xg,
                "x0own": x_rows[blk],
                "dinvb": dinvb,
                "idx1": t["idx1"],
                "dstl1": t["dstl1"],
                "idx2": t["idx2"],
                "dstl2": t["dstl2"],
            }
        )
    return row_of, GL, GH, G2, in_maps


_CACHE = {}


def _get_program(feat, src, dst, lambda_max):
    re_norm = float(
        np.float32(2.0) / np.asarray(lambda_max, np.float32).reshape(-1)[0]
    )
    key0 = (id(feat), id(src), id(dst), re_norm)
    if _CACHE.get("inkey") != key0:
        _CACHE["inmaps"] = _make_in_maps(feat, src, dst, re_norm)
        _CACHE["inkey"] = key0
    row_of, GL, GH, G2, in_maps = _CACHE["inmaps"]
    key = (GL, GH, G2, re_norm)
    if key not in _CACHE:
        _CACHE[key] = _build_bass(GL, GH, G2, re_norm)
    return _CACHE[key], in_maps, row_of


def kernel(feat, src, dst, lambda_max):
    from concourse.bass_utils import run_bass_kernel_spmd

    nc, in_maps, row_of = _get_program(feat, src, dst, lambda_max)
    res = run_bass_kernel_spmd(nc, in_maps, core_ids=list(range(NCORES)))
    kernel.last_exec_time_ns = res.exec_time_ns
    out_rows = np.concatenate(
        [res.results[c]["out"] for c in range(NCORES)], axis=0
    )
    return np.ascontiguousarray(out_rows[row_of[:N]])
